# revision 25
# baseline (speedup 1.0000x reference)
"""GCNConv (PyG-faithful, normalize=True, add_self_loops=True) on 8 Trainium2
NeuronCores via Bass/Tile.

Strategy (1D graph/data parallel):
  - Nodes are partitioned across the 8 cores (12500 rows each, padded to
    12544 = 98 blocks of 128).
  - Phase A: each core computes h_k = x_k @ W (fp32 matmuls), scales rows by
    dinv (symmetric GCN normalization, computed host-side from the edge
    index), casts to bf16 and AllGathers the scaled table
    g = dinv[:,None] * (x @ W) into every core's DRAM. The AllGather is
    split in two halves (first/second half of each shard) so phase-B
    gathers on chunks 0-1 can start while the second half is in flight.
  - Phase B: each core owns 1/8 of the destination nodes. Self-loop
    messages are the core's OWN h_shard rows: they are loaded per dst block
    with one affine DMA (no gather). Non-self edges are host-sorted by
    destination block; per 128-edge tile a dma_gather (SWDGE, 4 queues,
    calls split in halves and interleaved across queues to keep all rings
    fed) fetches g[src] rows (bf16). The one-hot selection tiles that map
    each edge to its dst-local row are host-precomputed (fp8) and streamed
    per window as one large-descriptor HWDGE DMA. TensorE segment-sums
    messages into a per-block PSUM accumulator; a final K=1 rank-1 matmul
    (sqrt(deg)[d] (x) bias[f]) folds the output bias into the accumulation
    so the epilogue is a single ScalarE copy scaled by dinv_dst.

  Per-(block, chunk) tile counts are computed from the actual edge data at
  call time (the program is compiled per call), maxed across cores so all 8
  cores run an identical (SPMD) program.
"""

import sys

if "/opt/trn_rl_repo" not in sys.path:
    sys.path.insert(0, "/opt/trn_rl_repo")

import numpy as np

P = 128          # partitions / tile edge count / feature dim
NCORES = 8
WBLK = 2         # blocks per window
CHUNKS = 4       # src chunks for int16 gather indices
GSPLIT = 1       # sub-calls per (window, chunk) gather section
DMA_SCRATCH = 16384  # SWDGE descriptor-ring carveout per partition (default;
                     # 48K/128K were tried and did not move the gather rate)

_PAD_DL = 160.0  # sentinel dst_local for pad edges -> all-zero sel column


def _pack(x, edge_index, weight, b):
    """Host-side preprocessing: sharding, normalization metadata, gather
    index packing, dst-local strips. All numpy, vectorized."""

    bias = b
    x = np.ascontiguousarray(np.asarray(x, dtype=np.float32))
    ei = np.asarray(edge_index)
    weight = np.ascontiguousarray(np.asarray(weight, dtype=np.float32))
    bias = np.asarray(bias, dtype=np.float32).reshape(-1)

    n, nin = x.shape
    nout = weight.shape[1]
    assert nin == P and nout == P, (nin, nout)
    assert n % NCORES == 0, n
    nb = n // NCORES                      # nodes per core (12500)
    blocks = (nb + P - 1) // P            # blocks per core (98)
    nbp = blocks * P                      # padded nodes per core (12544)
    half = nbp // 2                       # rows per AllGather half (6272)
    wblk = WBLK if blocks % WBLK == 0 else 1
    nwin = blocks // wblk                 # windows (14)
    chunk_rows = NCORES * half // 2       # rows per chunk (25088)
    assert chunk_rows < 32768, chunk_rows

    src0 = ei[0].astype(np.int64)
    dst0 = ei[1].astype(np.int64)

    # Degree-balanced relabeling: assign nodes to (core, block, lane) slots
    # snake-wise by in-degree so every block has a near-equal edge count.
    # This shrinks the cross-core max and the ceil-to-128 padding of the
    # per-(block, chunk) gather tiles. Src and dst sides (x, table, output)
    # share the permutation; the host unscrambles the output at the end.
    indeg = np.bincount(dst0, minlength=n)
    by_deg = np.argsort(-indeg, kind="stable")
    nslot = NCORES * blocks               # 784 (core, block) slots
    lastcap = nb - (blocks - 1) * P       # real lanes in the last block (84)
    slot_b = np.arange(nslot) % blocks
    newid = np.empty(n, np.int64)
    ptr = 0
    fwd = np.arange(nslot)
    for lane in range(P):
        active = fwd[(slot_b < blocks - 1) | (lane < lastcap)]
        if lane % 2 == 1:
            active = active[::-1]
        ids = (active // blocks) * nb + (active % blocks) * P + lane
        newid[by_deg[ptr : ptr + active.shape[0]]] = ids
        ptr += active.shape[0]
    assert ptr == n

    src = newid[src0]
    dst = newid[dst0]

    deg = np.bincount(dst, minlength=n).astype(np.float32) + 1.0
    dinv = 1.0 / np.sqrt(deg)
    sdeg = np.sqrt(deg)

    x = x[np.argsort(newid)]              # x row v_new = old row with newid==v_new

    m = src.shape[0]

    core = dst // nb
    dlc = dst - core * nb                 # dst local to core
    blk = dlc >> 7
    dl = (dlc & 127).astype(np.int64)
    # src table row in the split-AllGather layout: table A holds each
    # shard's rows [0, half), table B rows [half, nbp)
    kk = src // nb
    r = src % nb
    in_b = r >= half
    gh = kk * half + np.where(in_b, r - half, r)
    chunk = np.where(in_b, 2, 0) + gh // chunk_rows
    rel = (gh % chunk_rows).astype(np.int16)

    key = (core * blocks + blk) * CHUNKS + chunk
    # sort each section's edges by ascending src row (DRAM locality)
    order = np.lexsort((gh, key))
    karr = core[order]
    relarr = rel[order]
    dlarr = dl[order]
    gkey = key[order]

    counts = np.bincount(key, minlength=NCORES * blocks * CHUNKS).reshape(
        NCORES, blocks, CHUNKS
    )
    t_bc = -(-counts.max(axis=0) // P)    # [blocks, CHUNKS] tiles per slot

    # gather-tile layout: for w in windows: for c in chunks: for b in window
    tile_off = np.zeros((blocks, CHUNKS), np.int64)
    wbase = np.zeros(nwin + 1, np.int64)
    col = 0
    for w in range(nwin):
        wbase[w] = col
        for c in range(CHUNKS):
            for bb in range(w * wblk, (w + 1) * wblk):
                tile_off[bb, c] = col
                col += t_bc[bb, c]
    t_total = int(col)
    wbase[nwin] = col

    # scatter edges into per-core packed arrays
    gs = np.zeros(NCORES * blocks * CHUNKS, np.int64)
    gs[1:] = np.cumsum(counts.ravel())[:-1]
    rank = np.arange(m, dtype=np.int64) - gs[gkey]
    base_flat = (tile_off * P).ravel()    # same for all cores
    dest = base_flat[(gkey % (blocks * CHUNKS))] + rank

    idx_lin = np.zeros((NCORES, t_total * P), np.int16)
    dl_lin = np.full((NCORES, t_total * P), _PAD_DL, np.int16)
    idx_lin[karr, dest] = relarr
    dl_lin[karr, dest] = dlarr.astype(np.int16)

    # wrap-16 + replicate to 128 partitions for dma_gather idx layout
    l16 = t_total * P // 16
    idx_w = idx_lin.reshape(NCORES, l16, 16).transpose(0, 2, 1)  # [8,16,L16]
    idx_pack = np.ascontiguousarray(np.tile(idx_w, (1, NCORES, 1)))  # [8,128,L16]

    # host-precomputed one-hot sel tiles (fp8) for the gather slots, streamed
    # per window over HWDGE (building them on DVE stalls SWDGE descriptor
    # generation: DVE perf-mode ops hold the shared SBUF port pair that
    # GPSIMD needs to write DMA descriptors). sel[e, gt, d] = (dl[gt,e]==d).
    import ml_dtypes

    sel_pack = np.empty((NCORES, P, t_total * P), ml_dtypes.float8_e4m3)
    dgrid = np.arange(P, dtype=np.int16)[None, None, :]
    for k in range(NCORES):
        dlr = dl_lin[k].reshape(t_total, P)          # [gt, e]
        sel_k = dlr.T[:, :, None] == dgrid           # [e, gt, d] bool
        sel_pack[k] = sel_k.reshape(P, t_total * P).astype(ml_dtypes.float8_e4m3)

    # identity tile (fp8) for the self-loop matmuls
    ident_t = np.ascontiguousarray(
        np.eye(P, dtype=np.float32).astype(ml_dtypes.float8_e4m3)
    )

    # per-core xT, dinv, sqrt(deg) row strip (for the K=1 bias matmul)
    import ml_dtypes as _mld
    xt = np.zeros((NCORES, P, nbp), _mld.bfloat16)
    dinv_t = np.zeros((NCORES, P, blocks), np.float32)
    sdeg_t = np.zeros((NCORES, 1, nbp), np.float32)
    for k in range(NCORES):
        xs = x[k * nb : (k + 1) * nb]
        xt[k, :, :nb] = xs.T
        dv = np.zeros(nbp, np.float32)
        dv[:nb] = dinv[k * nb : (k + 1) * nb]
        dinv_t[k] = dv.reshape(blocks, P).T
        sv = np.zeros(nbp, np.float32)
        sv[:nb] = sdeg[k * nb : (k + 1) * nb]
        sdeg_t[k] = sv[None, :]
    bias_rep = np.ascontiguousarray(np.tile(bias[None, :], (P, 1)))

    meta = dict(
        n=n, nb=nb, blocks=blocks, nbp=nbp, nwin=nwin, wblk=wblk, newid=newid,
        chunk_rows=chunk_rows, half=half, t_bc=t_bc, tile_off=tile_off,
        wbase=wbase, t_total=t_total, l16=l16,
    )
    in_maps = [
        {
            "xt": xt[k],
            "w_in": weight,
            "bias": bias_rep,
            "dinv": dinv_t[k],
            "sdegT": sdeg_t[k],
            "idxp": idx_pack[k],
            "selp": sel_pack[k],
            "ident": ident_t,
        }
        for k in range(NCORES)
    ]
    return meta, in_maps


def _install_walrus_scratch_flag():
    """Make the walrus backend allocate the same enlarged dynamic-DMA
    scratch carveout that Bacc reserves (the ring size is a compiler flag,
    not a BIR attribute)."""
    from concourse import bass_utils

    if getattr(bass_utils, "_gcn_scratch_patched", None) == DMA_SCRATCH:
        return
    orig = bass_utils.get_walrus_args

    def patched(*args, **kwargs):
        return list(orig(*args, **kwargs)) + [
            f"--dynamic-dma-scratch-size-per-partition={DMA_SCRATCH}"
        ]

    bass_utils.get_walrus_args = patched
    bass_utils._gcn_scratch_patched = DMA_SCRATCH


def _build_program(meta, mbufs=12, dskew=8):
    from concourse import bass, bacc, mybir
    import concourse.tile as tile

    _install_walrus_scratch_flag()

    blocks = meta["blocks"]
    nbp = meta["nbp"]
    half = meta["half"]
    nwin = meta["nwin"]
    wblk = meta["wblk"]
    chunk_rows = meta["chunk_rows"]
    t_bc = meta["t_bc"]
    tile_off = meta["tile_off"]
    wbase = meta["wbase"]
    t_total = meta["t_total"]
    l16 = meta["l16"]
    jmax = int((wbase[1:] - wbase[:-1]).max())       # gather tiles per window
    smax = wblk + jmax                               # msg slots per window

    f32 = mybir.dt.float32
    bf16 = mybir.dt.bfloat16
    fp8 = mybir.dt.float8e4

    nc = bacc.Bacc(num_swdge_queues=4, dynamic_dma_scratch_size=DMA_SCRATCH)
    xt_in = nc.declare_dram_parameter("xt", [P, nbp], bf16, isOutput=False)
    w_in = nc.declare_dram_parameter("w_in", [P, P], f32, isOutput=False)
    bias_in = nc.declare_dram_parameter("bias", [P, P], f32, isOutput=False)
    dinv_in = nc.declare_dram_parameter("dinv", [P, blocks], f32, isOutput=False)
    sdegT_in = nc.declare_dram_parameter("sdegT", [1, nbp], f32, isOutput=False)
    idx_in = nc.declare_dram_parameter("idxp", [P, l16], mybir.dt.int16, isOutput=False)
    sel_in = nc.declare_dram_parameter("selp", [P, t_total * P], fp8, isOutput=False)
    ident_in = nc.declare_dram_parameter("ident", [P, P], fp8, isOutput=False)
    out_ext = nc.declare_dram_parameter("out", [nbp, P], f32, isOutput=True)

    h_shard = nc.dram_tensor("h_shard", [nbp, P], bf16)
    g_a = nc.dram_tensor("g_a", [NCORES * half, P], bf16, addr_space="Shared")
    g_b = nc.dram_tensor("g_b", [NCORES * half, P], bf16, addr_space="Shared")
    g_half = [g_a, g_a, g_b, g_b]

    with tile.TileContext(nc) as tc:
        with (
            tc.tile_pool(name="const", bufs=1) as cpool,
            tc.tile_pool(name="msgp", bufs=mbufs) as mpool,
            tc.tile_pool(name="selp", bufs=6) as spool,
            tc.tile_pool(name="idxp", bufs=mbufs) as ipool,
            tc.tile_pool(name="outp", bufs=3) as opool,
            tc.tile_pool(name="psB", bufs=6, space="PSUM") as psB,
        ):
            # constants / metadata loads
            w_sb = cpool.tile([P, P], f32, tag="w")
            nc.sync.dma_start(out=w_sb[:], in_=w_in[:])
            w_bf = cpool.tile([P, P], bf16, tag="wbf")
            nc.vector.tensor_scalar(
                out=w_bf[:], in0=w_sb[:], scalar1=1.0, scalar2=None,
                op0=mybir.AluOpType.mult,
            )
            bias_sb = cpool.tile([P, P], f32, tag="bias")
            nc.sync.dma_start(out=bias_sb[:], in_=bias_in[:])
            dinv_sb = cpool.tile([P, blocks], f32, tag="dinv")
            nc.sync.dma_start(out=dinv_sb[:], in_=dinv_in[:])
            ident_sb = cpool.tile([P, P], fp8, tag="ident")
            nc.sync.dma_start(out=ident_sb[:], in_=ident_in[:])

            # ---- phase A: h = x @ W, scale by dinv, cast bf16, allgather
            # (two halves: AG1 covers shard rows [0, half), AG2 the rest)
            with (
                tc.tile_pool(name="workA", bufs=2) as wpool,
                tc.tile_pool(name="psA", bufs=2, space="PSUM") as psA,
            ):
                nchunk = 14
                cw = nbp // nchunk        # nodes per chunk (896)
                tpc = cw // P             # tiles per chunk (7)
                for ch in range(nchunk):
                    xt_t = wpool.tile([P, cw], bf16, tag="xt")
                    nc.sync.dma_start(
                        out=xt_t[:], in_=xt_in[:, ch * cw : (ch + 1) * cw]
                    )
                    hbig = wpool.tile([P, tpc, P], bf16, tag="hbig")
                    for t in range(tpc):
                        ph = psA.tile([P, P], f32, tag="ph")
                        nc.tensor.matmul(
                            out=ph[:],
                            lhsT=xt_t[:, t * P : (t + 1) * P],
                            rhs=w_bf[:],
                            start=True,
                            stop=True,
                        )
                        gb = ch * tpc + t
                        nc.vector.tensor_scalar(
                            out=hbig[:, t, :],
                            in0=ph[:],
                            scalar1=dinv_sb[:, gb : gb + 1],
                            scalar2=None,
                            op0=mybir.AluOpType.mult,
                        )
                    nc.sync.dma_start(
                        out=h_shard[ch * cw : (ch + 1) * cw, :].rearrange(
                            "(t p) f -> p t f", p=P
                        ),
                        in_=hbig[:],
                    )
                    if ch == nchunk // 2 - 1:
                        nc.gpsimd.collective_compute(
                            "AllGather",
                            mybir.AluOpType.bypass,
                            replica_groups=[list(range(NCORES))],
                            ins=[h_shard[0:half, :]],
                            outs=[g_a[:]],
                        )
                nc.gpsimd.collective_compute(
                    "AllGather",
                    mybir.AluOpType.bypass,
                    replica_groups=[list(range(NCORES))],
                    ins=[h_shard[half:nbp, :]],
                    outs=[g_b[:]],
                )

            # ---- phase B: skewed pipeline. Chunk-0/1 gathers (table half A,
            # ready after AG1) issue for window w while chunk-2/3 gathers
            # (need AG2) issue for window w-DSKEW: AG2-blocked calls then
            # never clog the Pool engine's 4-deep wait queue while ready
            # chunk-0/1 work exists. Matmuls/epilogue run at w-DSKEW.
            state = {}

            def sections_of(w):
                secs = []
                for c in range(CHUNKS):
                    sec0 = None
                    seclen = 0
                    for bb in range(w * wblk, (w + 1) * wblk):
                        if t_bc[bb, c] > 0:
                            if sec0 is None:
                                sec0 = int(tile_off[bb, c])
                            seclen += int(t_bc[bb, c])
                    secs.append((sec0, seclen))
                return secs

            def emit_gather(w, c, idx_t, msg):
                sec0, seclen = state[w]["secs"][c]
                if seclen == 0:
                    return
                lo = sec0 - int(wbase[w])
                nc.gpsimd.dma_gather(
                    out_ap=msg[:, wblk + lo : wblk + lo + seclen, :],
                    in_ap=g_half[c][
                        (c % 2) * chunk_rows : (c % 2 + 1) * chunk_rows, :
                    ],
                    idxs_ap=idx_t[:, lo * 8 : (lo + seclen) * 8],
                    num_idxs=seclen * P,
                    num_idxs_reg=seclen * P,
                    elem_size=P,
                    single_packet=False,
                    queue_num=c,
                )

            for step in range(nwin + dskew):
                if step < nwin:
                    w = step
                    jsize = int(wbase[w + 1] - wbase[w])
                    idx_t = ipool.tile([P, jmax * 8], mybir.dt.int16, tag="idxw")
                    if jsize > 0:
                        nc.scalar.dma_start(
                            out=idx_t[:, : jsize * 8],
                            in_=idx_in[:, int(wbase[w]) * 8 : int(wbase[w + 1]) * 8],
                        )
                    msg = mpool.tile([P, smax, P], bf16, tag="msg")
                    nc.sync.dma_start(
                        out=msg[:, 0:wblk, :],
                        in_=h_shard[w * wblk * P : (w + 1) * wblk * P, :].rearrange(
                            "(j p) f -> p j f", p=P
                        ),
                    )
                    state[w] = dict(secs=sections_of(w), idx=idx_t, msg=msg, jsize=jsize)
                    emit_gather(w, 0, idx_t, msg)
                    emit_gather(w, 1, idx_t, msg)
                v = step - dskew
                if not (0 <= v < nwin):
                    continue
                st = state[v]
                idx_t, msg, jsize = st["idx"], st["msg"], st["jsize"]
                emit_gather(v, 2, idx_t, msg)
                emit_gather(v, 3, idx_t, msg)
                del state[v]
                selw = spool.tile([P, jmax, P], fp8, tag="selw")
                if jsize > 0:
                    nc.scalar.dma_start(
                        out=selw[:, :jsize, :],
                        in_=sel_in[:, int(wbase[v]) * P : int(wbase[v + 1]) * P],
                    )
                sdw = opool.tile([1, wblk * P], f32, tag="sdw")
                nc.sync.dma_start(
                    out=sdw[:],
                    in_=sdegT_in[:, v * wblk * P : (v + 1) * wblk * P],
                )
                osb_w = opool.tile([P, wblk, P], f32, tag="osbw")
                for j, bb in enumerate(range(v * wblk, (v + 1) * wblk)):
                    acc = psB.tile([P, P], f32, tag="acc")
                    nc.tensor.matmul(
                        out=acc[:],
                        lhsT=ident_sb[:],
                        rhs=msg[:, j, :],
                        start=True,
                        stop=False,
                    )
                    for c in range(CHUNKS):
                        tb = int(t_bc[bb, c])
                        for t in range(tb):
                            gt = int(tile_off[bb, c]) + t
                            scol = gt - int(wbase[v])
                            nc.tensor.matmul(
                                out=acc[:],
                                lhsT=selw[:, scol, :],
                                rhs=msg[:, wblk + scol, :],
                                start=False,
                                stop=False,
                            )
                    nc.tensor.matmul(
                        out=acc[:],
                        lhsT=sdw[:, j * P : (j + 1) * P],
                        rhs=bias_sb[0:1, :],
                        start=False,
                        stop=True,
                    )
                    nc.scalar.activation(
                        out=osb_w[:, j, :],
                        in_=acc[:],
                        func=mybir.ActivationFunctionType.Copy,
                        scale=dinv_sb[:, bb : bb + 1],
                    )
                nc.sync.dma_start(
                    out=out_ext[v * wblk * P : (v + 1) * wblk * P, :].rearrange(
                        "(j p) f -> p j f", p=P
                    ),
                    in_=osb_w[:],
                )

    nc.finalize()
    return nc


def _run(inputs, trace=False, trace_cores=None):
    from concourse.bass_utils import run_bass_kernel_spmd

    meta, in_maps = _pack(**inputs)
    nc = None
    for mb, dk in ((12, 8), (10, 8), (10, 6), (8, 6), (8, 4), (6, 4), (6, 2), (4, 2)):
        try:
            nc = _build_program(meta, mbufs=mb, dskew=dk)
            break
        except ValueError:
            continue
    assert nc is not None
    res = run_bass_kernel_spmd(
        nc,
        in_maps,
        list(range(NCORES)),
        trace=trace,
        trace_cores=trace_cores,
    )
    n, nb, nbp = meta["n"], meta["nb"], meta["nbp"]
    out_new = np.empty((n, P), np.float32)
    for k in range(NCORES):
        out_new[k * nb : (k + 1) * nb] = np.asarray(res.results[k]["out"])[:nb]
    out = out_new[meta["newid"]]
    return out, res


def kernel(x, edge_index, weight, b):
    out, _ = _run(dict(x=x, edge_index=edge_index, weight=weight, b=b))
    return out


if __name__ == "__main__":
    rng = np.random.default_rng(0)
    n, e = 100000, 1600000
    x = rng.standard_normal((n, P), dtype=np.float32)
    ei = rng.integers(0, n, (2, e)).astype(np.int64)
    w = (rng.standard_normal((P, P)) / np.sqrt(P)).astype(np.float32)
    bb = (rng.standard_normal(P) * 0.02).astype(np.float32)
    out = kernel(x, ei, w, bb)
    print("out", out.shape, out.dtype)


# revision 26
# speedup vs baseline: 1.0120x; 1.0120x over previous
"""GCNConv (PyG-faithful, normalize=True, add_self_loops=True) on 8 Trainium2
NeuronCores via Bass/Tile.

Strategy (1D graph/data parallel):
  - Nodes are partitioned across the 8 cores (12500 rows each, padded to
    12544 = 98 blocks of 128).
  - Phase A: each core computes h_k = x_k @ W (fp32 matmuls), scales rows by
    dinv (symmetric GCN normalization, computed host-side from the edge
    index), casts to bf16 and AllGathers the scaled table
    g = dinv[:,None] * (x @ W) into every core's DRAM. The AllGather is
    split in two halves (first/second half of each shard) so phase-B
    gathers on chunks 0-1 can start while the second half is in flight.
  - Phase B: each core owns 1/8 of the destination nodes. Self-loop
    messages are the core's OWN h_shard rows: they are loaded per dst block
    with one affine DMA (no gather). Non-self edges are host-sorted by
    destination block; per 128-edge tile a dma_gather (SWDGE, 4 queues,
    calls split in halves and interleaved across queues to keep all rings
    fed) fetches g[src] rows (bf16). The one-hot selection tiles that map
    each edge to its dst-local row are host-precomputed (fp8) and streamed
    per window as one large-descriptor HWDGE DMA. TensorE segment-sums
    messages into a per-block PSUM accumulator; a final K=1 rank-1 matmul
    (sqrt(deg)[d] (x) bias[f]) folds the output bias into the accumulation
    so the epilogue is a single ScalarE copy scaled by dinv_dst.

  Per-(block, chunk) tile counts are computed from the actual edge data at
  call time (the program is compiled per call), maxed across cores so all 8
  cores run an identical (SPMD) program.
"""

import sys

if "/opt/trn_rl_repo" not in sys.path:
    sys.path.insert(0, "/opt/trn_rl_repo")

import numpy as np

P = 128          # partitions / tile edge count / feature dim
NCORES = 8
WBLK = 2         # blocks per window
CHUNKS = 4       # src chunks for int16 gather indices
GSPLIT = 1       # sub-calls per (window, chunk) gather section
DMA_SCRATCH = 16384  # SWDGE descriptor-ring carveout per partition (default;
                     # 48K/128K were tried and did not move the gather rate)

_PAD_DL = 160.0  # sentinel dst_local for pad edges -> all-zero sel column


def _pack(x, edge_index, weight, b):
    """Host-side preprocessing: sharding, normalization metadata, gather
    index packing, dst-local strips. All numpy, vectorized."""

    bias = b
    x = np.ascontiguousarray(np.asarray(x, dtype=np.float32))
    ei = np.asarray(edge_index)
    weight = np.ascontiguousarray(np.asarray(weight, dtype=np.float32))
    bias = np.asarray(bias, dtype=np.float32).reshape(-1)

    n, nin = x.shape
    nout = weight.shape[1]
    assert nin == P and nout == P, (nin, nout)
    assert n % NCORES == 0, n
    nb = n // NCORES                      # nodes per core (12500)
    blocks = (nb + P - 1) // P            # blocks per core (98)
    nbp = blocks * P                      # padded nodes per core (12544)
    half = nbp // 2                       # rows per AllGather half (6272)
    wblk = WBLK if blocks % WBLK == 0 else 1
    nwin = blocks // wblk                 # windows (14)
    chunk_rows = NCORES * half // 2       # rows per chunk (25088)
    assert chunk_rows < 32768, chunk_rows

    src0 = ei[0].astype(np.int64)
    dst0 = ei[1].astype(np.int64)

    # Degree-balanced relabeling: assign nodes to (core, block, lane) slots
    # snake-wise by in-degree so every block has a near-equal edge count.
    # This shrinks the cross-core max and the ceil-to-128 padding of the
    # per-(block, chunk) gather tiles. Src and dst sides (x, table, output)
    # share the permutation; the host unscrambles the output at the end.
    indeg = np.bincount(dst0, minlength=n)
    by_deg = np.argsort(-indeg, kind="stable")
    nslot = NCORES * blocks               # 784 (core, block) slots
    lastcap = nb - (blocks - 1) * P       # real lanes in the last block (84)
    slot_b = np.arange(nslot) % blocks
    newid = np.empty(n, np.int64)
    ptr = 0
    fwd = np.arange(nslot)
    for lane in range(P):
        active = fwd[(slot_b < blocks - 1) | (lane < lastcap)]
        if lane % 2 == 1:
            active = active[::-1]
        ids = (active // blocks) * nb + (active % blocks) * P + lane
        newid[by_deg[ptr : ptr + active.shape[0]]] = ids
        ptr += active.shape[0]
    assert ptr == n

    src = newid[src0]
    dst = newid[dst0]

    deg = np.bincount(dst, minlength=n).astype(np.float32) + 1.0
    dinv = 1.0 / np.sqrt(deg)
    sdeg = np.sqrt(deg)

    x = x[np.argsort(newid)]              # x row v_new = old row with newid==v_new

    m = src.shape[0]

    core = dst // nb
    dlc = dst - core * nb                 # dst local to core
    blk = dlc >> 7
    dl = (dlc & 127).astype(np.int64)
    # src table row in the split-AllGather layout: table A holds each
    # shard's rows [0, half), table B rows [half, nbp)
    kk = src // nb
    r = src % nb
    in_b = r >= half
    gh = kk * half + np.where(in_b, r - half, r)
    chunk = np.where(in_b, 2, 0) + gh // chunk_rows
    rel = (gh % chunk_rows).astype(np.int16)

    key = (core * blocks + blk) * CHUNKS + chunk
    # sort each section's edges by ascending src row (DRAM locality)
    order = np.lexsort((gh, key))
    karr = core[order]
    relarr = rel[order]
    dlarr = dl[order]
    gkey = key[order]

    counts = np.bincount(key, minlength=NCORES * blocks * CHUNKS).reshape(
        NCORES, blocks, CHUNKS
    )
    t_bc = -(-counts.max(axis=0) // P)    # [blocks, CHUNKS] tiles per slot

    # gather-tile layout: for w in windows: for c in chunks: for b in window
    tile_off = np.zeros((blocks, CHUNKS), np.int64)
    wbase = np.zeros(nwin + 1, np.int64)
    col = 0
    for w in range(nwin):
        wbase[w] = col
        for c in range(CHUNKS):
            for bb in range(w * wblk, (w + 1) * wblk):
                tile_off[bb, c] = col
                col += t_bc[bb, c]
    t_total = int(col)
    wbase[nwin] = col

    # scatter edges into per-core packed arrays
    gs = np.zeros(NCORES * blocks * CHUNKS, np.int64)
    gs[1:] = np.cumsum(counts.ravel())[:-1]
    rank = np.arange(m, dtype=np.int64) - gs[gkey]
    base_flat = (tile_off * P).ravel()    # same for all cores
    dest = base_flat[(gkey % (blocks * CHUNKS))] + rank

    idx_lin = np.zeros((NCORES, t_total * P), np.int16)
    dl_lin = np.full((NCORES, t_total * P), _PAD_DL, np.int16)
    idx_lin[karr, dest] = relarr
    dl_lin[karr, dest] = dlarr.astype(np.int16)

    # wrap-16 + replicate to 128 partitions for dma_gather idx layout
    l16 = t_total * P // 16
    idx_w = idx_lin.reshape(NCORES, l16, 16).transpose(0, 2, 1)  # [8,16,L16]
    idx_pack = np.ascontiguousarray(np.tile(idx_w, (1, NCORES, 1)))  # [8,128,L16]

    # host-precomputed one-hot sel tiles (fp8) for the gather slots, streamed
    # per window over HWDGE (building them on DVE stalls SWDGE descriptor
    # generation: DVE perf-mode ops hold the shared SBUF port pair that
    # GPSIMD needs to write DMA descriptors). sel[e, gt, d] = (dl[gt,e]==d).
    import ml_dtypes

    sel_pack = np.empty((NCORES, P, t_total * P), ml_dtypes.float8_e4m3)
    dgrid = np.arange(P, dtype=np.int16)[None, None, :]
    for k in range(NCORES):
        dlr = dl_lin[k].reshape(t_total, P)          # [gt, e]
        sel_k = dlr.T[:, :, None] == dgrid           # [e, gt, d] bool
        sel_pack[k] = sel_k.reshape(P, t_total * P).astype(ml_dtypes.float8_e4m3)

    # identity tile (fp8) for the self-loop matmuls
    ident_t = np.ascontiguousarray(
        np.eye(P, dtype=np.float32).astype(ml_dtypes.float8_e4m3)
    )

    # per-core xT, dinv, sqrt(deg) row strip (for the K=1 bias matmul)
    import ml_dtypes as _mld
    xt = np.zeros((NCORES, P, nbp), _mld.bfloat16)
    dinv_t = np.zeros((NCORES, P, blocks), np.float32)
    sdeg_t = np.zeros((NCORES, 1, nbp), np.float32)
    for k in range(NCORES):
        xs = x[k * nb : (k + 1) * nb]
        xt[k, :, :nb] = xs.T
        dv = np.zeros(nbp, np.float32)
        dv[:nb] = dinv[k * nb : (k + 1) * nb]
        dinv_t[k] = dv.reshape(blocks, P).T
        sv = np.zeros(nbp, np.float32)
        sv[:nb] = sdeg[k * nb : (k + 1) * nb]
        sdeg_t[k] = sv[None, :]
    bias_rep = np.ascontiguousarray(np.tile(bias[None, :], (P, 1)))

    meta = dict(
        n=n, nb=nb, blocks=blocks, nbp=nbp, nwin=nwin, wblk=wblk, newid=newid,
        chunk_rows=chunk_rows, half=half, t_bc=t_bc, tile_off=tile_off,
        wbase=wbase, t_total=t_total, l16=l16,
    )
    in_maps = [
        {
            "xt": xt[k],
            "w_in": weight,
            "bias": bias_rep,
            "dinv": dinv_t[k],
            "sdegT": sdeg_t[k],
            "idxp": idx_pack[k],
            "selp": sel_pack[k],
            "ident": ident_t,
        }
        for k in range(NCORES)
    ]
    return meta, in_maps


def _install_walrus_scratch_flag():
    """Make the walrus backend allocate the same enlarged dynamic-DMA
    scratch carveout that Bacc reserves (the ring size is a compiler flag,
    not a BIR attribute)."""
    from concourse import bass_utils

    if getattr(bass_utils, "_gcn_scratch_patched", None) == DMA_SCRATCH:
        return
    orig = bass_utils.get_walrus_args

    def patched(*args, **kwargs):
        return list(orig(*args, **kwargs)) + [
            f"--dynamic-dma-scratch-size-per-partition={DMA_SCRATCH}"
        ]

    bass_utils.get_walrus_args = patched
    bass_utils._gcn_scratch_patched = DMA_SCRATCH


def _build_program(meta, mbufs=14):
    from concourse import bass, bacc, mybir
    import concourse.tile as tile

    _install_walrus_scratch_flag()

    blocks = meta["blocks"]
    nbp = meta["nbp"]
    half = meta["half"]
    nwin = meta["nwin"]
    wblk = meta["wblk"]
    chunk_rows = meta["chunk_rows"]
    t_bc = meta["t_bc"]
    tile_off = meta["tile_off"]
    wbase = meta["wbase"]
    t_total = meta["t_total"]
    l16 = meta["l16"]
    jmax = int((wbase[1:] - wbase[:-1]).max())       # gather tiles per window
    smax = wblk + jmax                               # msg slots per window

    f32 = mybir.dt.float32
    bf16 = mybir.dt.bfloat16
    fp8 = mybir.dt.float8e4

    nc = bacc.Bacc(num_swdge_queues=4, dynamic_dma_scratch_size=DMA_SCRATCH)
    xt_in = nc.declare_dram_parameter("xt", [P, nbp], bf16, isOutput=False)
    w_in = nc.declare_dram_parameter("w_in", [P, P], f32, isOutput=False)
    bias_in = nc.declare_dram_parameter("bias", [P, P], f32, isOutput=False)
    dinv_in = nc.declare_dram_parameter("dinv", [P, blocks], f32, isOutput=False)
    sdegT_in = nc.declare_dram_parameter("sdegT", [1, nbp], f32, isOutput=False)
    idx_in = nc.declare_dram_parameter("idxp", [P, l16], mybir.dt.int16, isOutput=False)
    sel_in = nc.declare_dram_parameter("selp", [P, t_total * P], fp8, isOutput=False)
    ident_in = nc.declare_dram_parameter("ident", [P, P], fp8, isOutput=False)
    out_ext = nc.declare_dram_parameter("out", [nbp, P], f32, isOutput=True)

    h_shard = nc.dram_tensor("h_shard", [nbp, P], bf16)
    g_a = nc.dram_tensor("g_a", [NCORES * half, P], bf16, addr_space="Shared")
    g_b = nc.dram_tensor("g_b", [NCORES * half, P], bf16, addr_space="Shared")
    g_half = [g_a, g_a, g_b, g_b]

    with tile.TileContext(nc) as tc:
        with (
            tc.tile_pool(name="const", bufs=1) as cpool,
            tc.tile_pool(name="msgp", bufs=mbufs) as mpool,
            tc.tile_pool(name="selp", bufs=6) as spool,
            tc.tile_pool(name="idxp", bufs=4) as ipool,
            tc.tile_pool(name="outp", bufs=3) as opool,
            tc.tile_pool(name="psB", bufs=6, space="PSUM") as psB,
        ):
            # constants / metadata loads
            w_sb = cpool.tile([P, P], f32, tag="w")
            nc.sync.dma_start(out=w_sb[:], in_=w_in[:])
            w_bf = cpool.tile([P, P], bf16, tag="wbf")
            nc.vector.tensor_scalar(
                out=w_bf[:], in0=w_sb[:], scalar1=1.0, scalar2=None,
                op0=mybir.AluOpType.mult,
            )
            bias_sb = cpool.tile([P, P], f32, tag="bias")
            nc.sync.dma_start(out=bias_sb[:], in_=bias_in[:])
            dinv_sb = cpool.tile([P, blocks], f32, tag="dinv")
            nc.sync.dma_start(out=dinv_sb[:], in_=dinv_in[:])
            ident_sb = cpool.tile([P, P], fp8, tag="ident")
            nc.sync.dma_start(out=ident_sb[:], in_=ident_in[:])

            # ---- phase A: h = x @ W, scale by dinv, cast bf16, allgather
            # (two halves: AG1 covers shard rows [0, half), AG2 the rest)
            with (
                tc.tile_pool(name="workA", bufs=2) as wpool,
                tc.tile_pool(name="psA", bufs=2, space="PSUM") as psA,
            ):
                nchunk = 14
                cw = nbp // nchunk        # nodes per chunk (896)
                tpc = cw // P             # tiles per chunk (7)
                for ch in range(nchunk):
                    xt_t = wpool.tile([P, cw], bf16, tag="xt")
                    nc.sync.dma_start(
                        out=xt_t[:], in_=xt_in[:, ch * cw : (ch + 1) * cw]
                    )
                    hbig = wpool.tile([P, tpc, P], bf16, tag="hbig")
                    for t in range(tpc):
                        ph = psA.tile([P, P], f32, tag="ph")
                        nc.tensor.matmul(
                            out=ph[:],
                            lhsT=xt_t[:, t * P : (t + 1) * P],
                            rhs=w_bf[:],
                            start=True,
                            stop=True,
                        )
                        gb = ch * tpc + t
                        nc.vector.tensor_scalar(
                            out=hbig[:, t, :],
                            in0=ph[:],
                            scalar1=dinv_sb[:, gb : gb + 1],
                            scalar2=None,
                            op0=mybir.AluOpType.mult,
                        )
                    nc.sync.dma_start(
                        out=h_shard[ch * cw : (ch + 1) * cw, :].rearrange(
                            "(t p) f -> p t f", p=P
                        ),
                        in_=hbig[:],
                    )
                    if ch == nchunk // 2 - 1:
                        nc.gpsimd.collective_compute(
                            "AllGather",
                            mybir.AluOpType.bypass,
                            replica_groups=[list(range(NCORES))],
                            ins=[h_shard[0:half, :]],
                            outs=[g_a[:]],
                        )
                nc.gpsimd.collective_compute(
                    "AllGather",
                    mybir.AluOpType.bypass,
                    replica_groups=[list(range(NCORES))],
                    ins=[h_shard[half:nbp, :]],
                    outs=[g_b[:]],
                )

            # ---- phase B: self tiles + gather + on-chip one-hot segment
            # matmul per dst block
            for w in range(nwin):
                jsize = int(wbase[w + 1] - wbase[w])
                ssize = wblk + jsize
                # stream this window's gather indices (scalar HWDGE queue)
                idx_t = ipool.tile([P, jmax * 8], mybir.dt.int16, tag="idxw")
                if jsize > 0:
                    nc.scalar.dma_start(
                        out=idx_t[:, : jsize * 8],
                        in_=idx_in[:, int(wbase[w]) * 8 : int(wbase[w + 1]) * 8],
                    )
                # stream this window's sel tiles (one contiguous HWDGE DMA,
                # 128 large descriptors)
                selw = spool.tile([P, jmax, P], fp8, tag="selw")
                if jsize > 0:
                    nc.scalar.dma_start(
                        out=selw[:, :jsize, :],
                        in_=sel_in[
                            :, int(wbase[w]) * P : int(wbase[w + 1]) * P
                        ],
                    )
                msg = mpool.tile([P, smax, P], bf16, tag="msg")
                # self tiles: the window's own (already dinv-scaled) rows
                nc.sync.dma_start(
                    out=msg[:, 0:wblk, :],
                    in_=h_shard[w * wblk * P : (w + 1) * wblk * P, :].rearrange(
                        "(j p) f -> p j f", p=P
                    ),
                )
                # gather sections, split into GSPLIT sub-calls interleaved
                # across the 4 queues so no ring-full stall starves a queue
                secs = []
                for c in range(CHUNKS):
                    sec0 = None
                    seclen = 0
                    for bb in range(w * wblk, (w + 1) * wblk):
                        if t_bc[bb, c] > 0:
                            if sec0 is None:
                                sec0 = int(tile_off[bb, c])
                            seclen += int(t_bc[bb, c])
                    secs.append((sec0, seclen))
                for s in range(GSPLIT):
                    for c in range(CHUNKS):
                        sec0, seclen = secs[c]
                        if seclen == 0:
                            continue
                        a = seclen * s // GSPLIT
                        e = seclen * (s + 1) // GSPLIT
                        if e == a:
                            continue
                        t0c = sec0 + a
                        lo = t0c - int(wbase[w])
                        nc.gpsimd.dma_gather(
                            out_ap=msg[:, wblk + lo : wblk + lo + (e - a), :],
                            in_ap=g_half[c][
                                (c % 2) * chunk_rows : (c % 2 + 1) * chunk_rows, :
                            ],
                            idxs_ap=idx_t[:, lo * 8 : (lo + (e - a)) * 8],
                            num_idxs=(e - a) * P,
                            num_idxs_reg=(e - a) * P,
                            elem_size=P,
                            single_packet=False,
                            queue_num=c,
                        )
                sdw = opool.tile([1, wblk * P], f32, tag="sdw")
                nc.sync.dma_start(
                    out=sdw[:],
                    in_=sdegT_in[:, w * wblk * P : (w + 1) * wblk * P],
                )
                osb_w = opool.tile([P, wblk, P], f32, tag="osbw")
                for j, bb in enumerate(range(w * wblk, (w + 1) * wblk)):
                    acc = psB.tile([P, P], f32, tag="acc")
                    # self tile opens the accumulation (identity sel)
                    nc.tensor.matmul(
                        out=acc[:],
                        lhsT=ident_sb[:],
                        rhs=msg[:, j, :],
                        start=True,
                        stop=False,
                    )
                    for c in range(CHUNKS):
                        tb = int(t_bc[bb, c])
                        for t in range(tb):
                            gt = int(tile_off[bb, c]) + t
                            scol = gt - int(wbase[w])
                            nc.tensor.matmul(
                                out=acc[:],
                                lhsT=selw[:, scol, :],
                                rhs=msg[:, wblk + scol, :],
                                start=False,
                                stop=False,
                            )
                    # fold bias: acc += sqrt(deg)[d] (x) bias[f] (K=1 matmul);
                    # the dinv_dst epilogue scale turns it into exactly +bias
                    nc.tensor.matmul(
                        out=acc[:],
                        lhsT=sdw[:, j * P : (j + 1) * P],
                        rhs=bias_sb[0:1, :],
                        start=False,
                        stop=True,
                    )
                    # epilogue: scale by dinv_dst on ScalarE (PSUM -> SBUF)
                    nc.scalar.activation(
                        out=osb_w[:, j, :],
                        in_=acc[:],
                        func=mybir.ActivationFunctionType.Copy,
                        scale=dinv_sb[:, bb : bb + 1],
                    )
                nc.sync.dma_start(
                    out=out_ext[w * wblk * P : (w + 1) * wblk * P, :].rearrange(
                        "(j p) f -> p j f", p=P
                    ),
                    in_=osb_w[:],
                )

    nc.finalize()
    return nc


def _run(inputs, trace=False, trace_cores=None):
    from concourse.bass_utils import run_bass_kernel_spmd

    meta, in_maps = _pack(**inputs)
    nc = None
    for mb in (14, 12, 10, 8, 6, 4, 3, 2):
        try:
            nc = _build_program(meta, mbufs=mb)
            break
        except ValueError:
            continue
    assert nc is not None
    res = run_bass_kernel_spmd(
        nc,
        in_maps,
        list(range(NCORES)),
        trace=trace,
        trace_cores=trace_cores,
    )
    n, nb, nbp = meta["n"], meta["nb"], meta["nbp"]
    out_new = np.empty((n, P), np.float32)
    for k in range(NCORES):
        out_new[k * nb : (k + 1) * nb] = np.asarray(res.results[k]["out"])[:nb]
    out = out_new[meta["newid"]]
    return out, res


def kernel(x, edge_index, weight, b):
    out, _ = _run(dict(x=x, edge_index=edge_index, weight=weight, b=b))
    return out


if __name__ == "__main__":
    rng = np.random.default_rng(0)
    n, e = 100000, 1600000
    x = rng.standard_normal((n, P), dtype=np.float32)
    ei = rng.integers(0, n, (2, e)).astype(np.int64)
    w = (rng.standard_normal((P, P)) / np.sqrt(P)).astype(np.float32)
    bb = (rng.standard_normal(P) * 0.02).astype(np.float32)
    out = kernel(x, ei, w, bb)
    print("out", out.shape, out.dtype)


# revision 28
# speedup vs baseline: 1.1867x; 1.1726x over previous
"""GCNConv (PyG-faithful, normalize=True, add_self_loops=True) on 8 Trainium2
NeuronCores via Bass/Tile.

Strategy (1D graph/data parallel):
  - Nodes are partitioned across the 8 cores (12500 rows each, padded to
    12544 = 98 blocks of 128).
  - Phase A: each core computes h_k = x_k @ W (fp32 matmuls), scales rows by
    dinv (symmetric GCN normalization, computed host-side from the edge
    index), casts to bf16 and AllGathers the scaled table
    g = dinv[:,None] * (x @ W) into every core's DRAM. The AllGather is
    split in two halves (first/second half of each shard) so phase-B
    gathers on chunks 0-1 can start while the second half is in flight.
  - Phase B: each core owns 1/8 of the destination nodes. Self-loop
    messages are the core's OWN h_shard rows: they are loaded per dst block
    with one affine DMA (no gather). Non-self edges are host-sorted by
    destination block; per 128-edge tile a dma_gather (SWDGE, 4 queues,
    calls split in halves and interleaved across queues to keep all rings
    fed) fetches g[src] rows (bf16). The one-hot selection tiles that map
    each edge to its dst-local row are host-precomputed (fp8) and streamed
    per window as one large-descriptor HWDGE DMA. TensorE segment-sums
    messages into a per-block PSUM accumulator; a final K=1 rank-1 matmul
    (sqrt(deg)[d] (x) bias[f]) folds the output bias into the accumulation
    so the epilogue is a single ScalarE copy scaled by dinv_dst.

  Per-(block, chunk) tile counts are computed from the actual edge data at
  call time (the program is compiled per call), maxed across cores so all 8
  cores run an identical (SPMD) program.
"""

import sys

if "/opt/trn_rl_repo" not in sys.path:
    sys.path.insert(0, "/opt/trn_rl_repo")

import numpy as np

P = 128          # partitions / tile edge count / feature dim
NCORES = 8
WBLK = 2         # blocks per window
CHUNKS = 4       # src chunks for int16 gather indices
GSPLIT = 1       # sub-calls per (window, chunk) gather section
DMA_SCRATCH = 16384  # SWDGE descriptor-ring carveout per partition (default;
                     # 48K/128K were tried and did not move the gather rate)

_PAD_DL = 160.0  # sentinel dst_local for pad edges -> all-zero sel column


def _pack(x, edge_index, weight, b):
    """Host-side preprocessing: sharding, normalization metadata, gather
    index packing, dst-local strips. All numpy, vectorized."""

    bias = b
    x = np.ascontiguousarray(np.asarray(x, dtype=np.float32))
    ei = np.asarray(edge_index)
    weight = np.ascontiguousarray(np.asarray(weight, dtype=np.float32))
    bias = np.asarray(bias, dtype=np.float32).reshape(-1)

    n, nin = x.shape
    nout = weight.shape[1]
    assert nin == P and nout == P, (nin, nout)
    assert n % NCORES == 0, n
    nb = n // NCORES                      # nodes per core (12500)
    blocks = (nb + P - 1) // P            # blocks per core (98)
    nbp = blocks * P                      # padded nodes per core (12544)
    # asymmetric AllGather halves: half A gets ~54% of rows so the early-
    # starting chunk-0/1 gather queues (skewed pipeline, below) carry
    # proportionally more work and all four queues finish together
    ha = 6784                             # shard rows in half A (54.1%)
    hb = nbp - ha                         # shard rows in half B (5760)
    wblk = WBLK if blocks % WBLK == 0 else 1
    nwin = blocks // wblk                 # windows (14)
    crows_a = NCORES * ha // 2            # rows per A chunk (27136)
    crows_b = NCORES * hb // 2            # rows per B chunk (23040)
    assert crows_a < 32768 and crows_b < 32768, (crows_a, crows_b)

    src0 = ei[0].astype(np.int64)
    dst0 = ei[1].astype(np.int64)

    # Degree-balanced relabeling: assign nodes to (core, block, lane) slots
    # snake-wise by in-degree so every block has a near-equal edge count.
    # This shrinks the cross-core max and the ceil-to-128 padding of the
    # per-(block, chunk) gather tiles. Src and dst sides (x, table, output)
    # share the permutation; the host unscrambles the output at the end.
    indeg = np.bincount(dst0, minlength=n)
    by_deg = np.argsort(-indeg, kind="stable")
    nslot = NCORES * blocks               # 784 (core, block) slots
    lastcap = nb - (blocks - 1) * P       # real lanes in the last block (84)
    slot_b = np.arange(nslot) % blocks
    newid = np.empty(n, np.int64)
    ptr = 0
    fwd = np.arange(nslot)
    for lane in range(P):
        active = fwd[(slot_b < blocks - 1) | (lane < lastcap)]
        if lane % 2 == 1:
            active = active[::-1]
        ids = (active // blocks) * nb + (active % blocks) * P + lane
        newid[by_deg[ptr : ptr + active.shape[0]]] = ids
        ptr += active.shape[0]
    assert ptr == n

    src = newid[src0]
    dst = newid[dst0]

    deg = np.bincount(dst, minlength=n).astype(np.float32) + 1.0
    dinv = 1.0 / np.sqrt(deg)
    sdeg = np.sqrt(deg)

    x = x[np.argsort(newid)]              # x row v_new = old row with newid==v_new

    m = src.shape[0]

    core = dst // nb
    dlc = dst - core * nb                 # dst local to core
    blk = dlc >> 7
    dl = (dlc & 127).astype(np.int64)
    # src table row in the split-AllGather layout: table A holds each
    # shard's rows [0, ha), table B rows [ha, nbp)
    kk = src // nb
    r = src % nb
    in_b = r >= ha
    gh = np.where(in_b, kk * hb + (r - ha), kk * ha + r)
    crw = np.where(in_b, crows_b, crows_a)
    chunk = np.where(in_b, 2, 0) + gh // crw
    rel = (gh % crw).astype(np.int16)

    key = (core * blocks + blk) * CHUNKS + chunk
    # sort each section's edges by ascending src row (DRAM locality)
    order = np.lexsort((gh, key))
    karr = core[order]
    relarr = rel[order]
    dlarr = dl[order]
    gkey = key[order]

    counts = np.bincount(key, minlength=NCORES * blocks * CHUNKS).reshape(
        NCORES, blocks, CHUNKS
    )
    t_bc = -(-counts.max(axis=0) // P)    # [blocks, CHUNKS] tiles per slot

    # gather-tile layout: for w in windows: for c in chunks: for b in window
    tile_off = np.zeros((blocks, CHUNKS), np.int64)
    wbase = np.zeros(nwin + 1, np.int64)
    col = 0
    for w in range(nwin):
        wbase[w] = col
        for c in range(CHUNKS):
            for bb in range(w * wblk, (w + 1) * wblk):
                tile_off[bb, c] = col
                col += t_bc[bb, c]
    t_total = int(col)
    wbase[nwin] = col

    # scatter edges into per-core packed arrays
    gs = np.zeros(NCORES * blocks * CHUNKS, np.int64)
    gs[1:] = np.cumsum(counts.ravel())[:-1]
    rank = np.arange(m, dtype=np.int64) - gs[gkey]
    base_flat = (tile_off * P).ravel()    # same for all cores
    dest = base_flat[(gkey % (blocks * CHUNKS))] + rank

    idx_lin = np.zeros((NCORES, t_total * P), np.int16)
    dl_lin = np.full((NCORES, t_total * P), _PAD_DL, np.int16)
    idx_lin[karr, dest] = relarr
    dl_lin[karr, dest] = dlarr.astype(np.int16)

    # wrap-16 + replicate to 128 partitions for dma_gather idx layout
    l16 = t_total * P // 16
    idx_w = idx_lin.reshape(NCORES, l16, 16).transpose(0, 2, 1)  # [8,16,L16]
    idx_pack = np.ascontiguousarray(np.tile(idx_w, (1, NCORES, 1)))  # [8,128,L16]

    # host-precomputed one-hot sel tiles (fp8) for the gather slots, streamed
    # per window over HWDGE (building them on DVE stalls SWDGE descriptor
    # generation: DVE perf-mode ops hold the shared SBUF port pair that
    # GPSIMD needs to write DMA descriptors). sel[e, gt, d] = (dl[gt,e]==d).
    import ml_dtypes

    sel_pack = np.empty((NCORES, P, t_total * P), ml_dtypes.float8_e4m3)
    dgrid = np.arange(P, dtype=np.int16)[None, None, :]
    for k in range(NCORES):
        dlr = dl_lin[k].reshape(t_total, P)          # [gt, e]
        sel_k = dlr.T[:, :, None] == dgrid           # [e, gt, d] bool
        sel_pack[k] = sel_k.reshape(P, t_total * P).astype(ml_dtypes.float8_e4m3)

    # identity tile (fp8) for the self-loop matmuls
    ident_t = np.ascontiguousarray(
        np.eye(P, dtype=np.float32).astype(ml_dtypes.float8_e4m3)
    )

    # per-core xT, dinv, sqrt(deg) row strip (for the K=1 bias matmul)
    import ml_dtypes as _mld
    xt = np.zeros((NCORES, P, nbp), _mld.bfloat16)
    dinv_t = np.zeros((NCORES, P, blocks), np.float32)
    sdeg_t = np.zeros((NCORES, 1, nbp), np.float32)
    for k in range(NCORES):
        xs = x[k * nb : (k + 1) * nb]
        xt[k, :, :nb] = xs.T
        dv = np.zeros(nbp, np.float32)
        dv[:nb] = dinv[k * nb : (k + 1) * nb]
        dinv_t[k] = dv.reshape(blocks, P).T
        sv = np.zeros(nbp, np.float32)
        sv[:nb] = sdeg[k * nb : (k + 1) * nb]
        sdeg_t[k] = sv[None, :]
    bias_rep = np.ascontiguousarray(np.tile(bias[None, :], (P, 1)))

    meta = dict(
        n=n, nb=nb, blocks=blocks, nbp=nbp, nwin=nwin, wblk=wblk, newid=newid,
        ha=ha, hb=hb, crows_a=crows_a, crows_b=crows_b, t_bc=t_bc, tile_off=tile_off,
        wbase=wbase, t_total=t_total, l16=l16,
    )
    in_maps = [
        {
            "xt": xt[k],
            "w_in": weight,
            "bias": bias_rep,
            "dinv": dinv_t[k],
            "sdegT": sdeg_t[k],
            "idxp": idx_pack[k],
            "selp": sel_pack[k],
            "ident": ident_t,
        }
        for k in range(NCORES)
    ]
    return meta, in_maps


def _install_walrus_scratch_flag():
    """Make the walrus backend allocate the same enlarged dynamic-DMA
    scratch carveout that Bacc reserves (the ring size is a compiler flag,
    not a BIR attribute)."""
    from concourse import bass_utils

    if getattr(bass_utils, "_gcn_scratch_patched", None) == DMA_SCRATCH:
        return
    orig = bass_utils.get_walrus_args

    def patched(*args, **kwargs):
        return list(orig(*args, **kwargs)) + [
            f"--dynamic-dma-scratch-size-per-partition={DMA_SCRATCH}"
        ]

    bass_utils.get_walrus_args = patched
    bass_utils._gcn_scratch_patched = DMA_SCRATCH


def _build_program(meta, mbufs=12, dskew=8):
    from concourse import bass, bacc, mybir
    import concourse.tile as tile

    _install_walrus_scratch_flag()

    blocks = meta["blocks"]
    nbp = meta["nbp"]
    ha, hb = meta["ha"], meta["hb"]
    crows = [meta["crows_a"], meta["crows_a"], meta["crows_b"], meta["crows_b"]]
    nwin = meta["nwin"]
    wblk = meta["wblk"]
    t_bc = meta["t_bc"]
    tile_off = meta["tile_off"]
    wbase = meta["wbase"]
    t_total = meta["t_total"]
    l16 = meta["l16"]
    jmax = int((wbase[1:] - wbase[:-1]).max())       # gather tiles per window
    smax = wblk + jmax                               # msg slots per window

    f32 = mybir.dt.float32
    bf16 = mybir.dt.bfloat16
    fp8 = mybir.dt.float8e4

    nc = bacc.Bacc(num_swdge_queues=4, dynamic_dma_scratch_size=DMA_SCRATCH)
    xt_in = nc.declare_dram_parameter("xt", [P, nbp], bf16, isOutput=False)
    w_in = nc.declare_dram_parameter("w_in", [P, P], f32, isOutput=False)
    bias_in = nc.declare_dram_parameter("bias", [P, P], f32, isOutput=False)
    dinv_in = nc.declare_dram_parameter("dinv", [P, blocks], f32, isOutput=False)
    sdegT_in = nc.declare_dram_parameter("sdegT", [1, nbp], f32, isOutput=False)
    idx_in = nc.declare_dram_parameter("idxp", [P, l16], mybir.dt.int16, isOutput=False)
    sel_in = nc.declare_dram_parameter("selp", [P, t_total * P], fp8, isOutput=False)
    ident_in = nc.declare_dram_parameter("ident", [P, P], fp8, isOutput=False)
    out_ext = nc.declare_dram_parameter("out", [nbp, P], f32, isOutput=True)

    h_shard = nc.dram_tensor("h_shard", [nbp, P], bf16)
    g_a = nc.dram_tensor("g_a", [NCORES * ha, P], bf16, addr_space="Shared")
    g_b = nc.dram_tensor("g_b", [NCORES * hb, P], bf16, addr_space="Shared")
    g_half = [g_a, g_a, g_b, g_b]

    with tile.TileContext(nc) as tc:
        with (
            tc.tile_pool(name="const", bufs=1) as cpool,
            tc.tile_pool(name="msgp", bufs=mbufs) as mpool,
            tc.tile_pool(name="selp", bufs=6) as spool,
            tc.tile_pool(name="idxp", bufs=mbufs) as ipool,
            tc.tile_pool(name="outp", bufs=3) as opool,
            tc.tile_pool(name="psB", bufs=6, space="PSUM") as psB,
        ):
            # constants / metadata loads
            w_sb = cpool.tile([P, P], f32, tag="w")
            nc.sync.dma_start(out=w_sb[:], in_=w_in[:])
            w_bf = cpool.tile([P, P], bf16, tag="wbf")
            nc.vector.tensor_scalar(
                out=w_bf[:], in0=w_sb[:], scalar1=1.0, scalar2=None,
                op0=mybir.AluOpType.mult,
            )
            bias_sb = cpool.tile([P, P], f32, tag="bias")
            nc.sync.dma_start(out=bias_sb[:], in_=bias_in[:])
            dinv_sb = cpool.tile([P, blocks], f32, tag="dinv")
            nc.sync.dma_start(out=dinv_sb[:], in_=dinv_in[:])
            ident_sb = cpool.tile([P, P], fp8, tag="ident")
            nc.sync.dma_start(out=ident_sb[:], in_=ident_in[:])

            # ---- phase A: h = x @ W, scale by dinv, cast bf16, allgather
            # (two halves: AG1 covers shard rows [0, half), AG2 the rest)
            with (
                tc.tile_pool(name="workA", bufs=2) as wpool,
                tc.tile_pool(name="psA", bufs=2, space="PSUM") as psA,
            ):
                nchunk = 14
                cw = nbp // nchunk        # nodes per chunk (896)
                tpc = cw // P             # tiles per chunk (7)
                for ch in range(nchunk):
                    xt_t = wpool.tile([P, cw], bf16, tag="xt")
                    nc.sync.dma_start(
                        out=xt_t[:], in_=xt_in[:, ch * cw : (ch + 1) * cw]
                    )
                    hbig = wpool.tile([P, tpc, P], bf16, tag="hbig")
                    for t in range(tpc):
                        ph = psA.tile([P, P], f32, tag="ph")
                        nc.tensor.matmul(
                            out=ph[:],
                            lhsT=xt_t[:, t * P : (t + 1) * P],
                            rhs=w_bf[:],
                            start=True,
                            stop=True,
                        )
                        gb = ch * tpc + t
                        nc.vector.tensor_scalar(
                            out=hbig[:, t, :],
                            in0=ph[:],
                            scalar1=dinv_sb[:, gb : gb + 1],
                            scalar2=None,
                            op0=mybir.AluOpType.mult,
                        )
                    nc.sync.dma_start(
                        out=h_shard[ch * cw : (ch + 1) * cw, :].rearrange(
                            "(t p) f -> p t f", p=P
                        ),
                        in_=hbig[:],
                    )
                    if (ch + 1) * cw >= ha > ch * cw:
                        nc.gpsimd.collective_compute(
                            "AllGather",
                            mybir.AluOpType.bypass,
                            replica_groups=[list(range(NCORES))],
                            ins=[h_shard[0:ha, :]],
                            outs=[g_a[:]],
                        )
                nc.gpsimd.collective_compute(
                    "AllGather",
                    mybir.AluOpType.bypass,
                    replica_groups=[list(range(NCORES))],
                    ins=[h_shard[ha:nbp, :]],
                    outs=[g_b[:]],
                )

            # ---- phase B: skewed pipeline. Chunk-0/1 gathers (table half A,
            # ready after AG1) issue for window w while chunk-2/3 gathers
            # (need AG2) issue for window w-dskew: AG2-blocked calls then
            # never clog the Pool engine's 4-deep wait queue while ready
            # chunk-0/1 work exists. Matmuls/epilogue run at w-dskew.
            state = {}

            def sections_of(w):
                secs = []
                for c in range(CHUNKS):
                    sec0 = None
                    seclen = 0
                    for bb in range(w * wblk, (w + 1) * wblk):
                        if t_bc[bb, c] > 0:
                            if sec0 is None:
                                sec0 = int(tile_off[bb, c])
                            seclen += int(t_bc[bb, c])
                    secs.append((sec0, seclen))
                return secs

            def emit_gather(w, c, idx_t, msg):
                sec0, seclen = state[w]["secs"][c]
                if seclen == 0:
                    return
                lo = sec0 - int(wbase[w])
                nc.gpsimd.dma_gather(
                    out_ap=msg[:, wblk + lo : wblk + lo + seclen, :],
                    in_ap=g_half[c][
                        (c % 2) * crows[c] : (c % 2 + 1) * crows[c], :
                    ],
                    idxs_ap=idx_t[:, lo * 8 : (lo + seclen) * 8],
                    num_idxs=seclen * P,
                    num_idxs_reg=seclen * P,
                    elem_size=P,
                    single_packet=False,
                    queue_num=c,
                )

            for step in range(nwin + dskew):
                if step < nwin:
                    w = step
                    jsize = int(wbase[w + 1] - wbase[w])
                    idx_t = ipool.tile([P, jmax * 8], mybir.dt.int16, tag="idxw")
                    if jsize > 0:
                        nc.scalar.dma_start(
                            out=idx_t[:, : jsize * 8],
                            in_=idx_in[:, int(wbase[w]) * 8 : int(wbase[w + 1]) * 8],
                        )
                    msg = mpool.tile([P, smax, P], bf16, tag="msg")
                    nc.sync.dma_start(
                        out=msg[:, 0:wblk, :],
                        in_=h_shard[w * wblk * P : (w + 1) * wblk * P, :].rearrange(
                            "(j p) f -> p j f", p=P
                        ),
                    )
                    state[w] = dict(secs=sections_of(w), idx=idx_t, msg=msg, jsize=jsize)
                    emit_gather(w, 0, idx_t, msg)
                    emit_gather(w, 1, idx_t, msg)
                v = step - dskew
                if not (0 <= v < nwin):
                    continue
                st = state[v]
                idx_t, msg, jsize = st["idx"], st["msg"], st["jsize"]
                emit_gather(v, 2, idx_t, msg)
                emit_gather(v, 3, idx_t, msg)
                del state[v]
                selw = spool.tile([P, jmax, P], fp8, tag="selw")
                if jsize > 0:
                    nc.scalar.dma_start(
                        out=selw[:, :jsize, :],
                        in_=sel_in[:, int(wbase[v]) * P : int(wbase[v + 1]) * P],
                    )
                sdw = opool.tile([1, wblk * P], f32, tag="sdw")
                nc.sync.dma_start(
                    out=sdw[:],
                    in_=sdegT_in[:, v * wblk * P : (v + 1) * wblk * P],
                )
                osb_w = opool.tile([P, wblk, P], f32, tag="osbw")
                for j, bb in enumerate(range(v * wblk, (v + 1) * wblk)):
                    acc = psB.tile([P, P], f32, tag="acc")
                    nc.tensor.matmul(
                        out=acc[:],
                        lhsT=ident_sb[:],
                        rhs=msg[:, j, :],
                        start=True,
                        stop=False,
                    )
                    for c in range(CHUNKS):
                        tb = int(t_bc[bb, c])
                        for t in range(tb):
                            gt = int(tile_off[bb, c]) + t
                            scol = gt - int(wbase[v])
                            nc.tensor.matmul(
                                out=acc[:],
                                lhsT=selw[:, scol, :],
                                rhs=msg[:, wblk + scol, :],
                                start=False,
                                stop=False,
                            )
                    nc.tensor.matmul(
                        out=acc[:],
                        lhsT=sdw[:, j * P : (j + 1) * P],
                        rhs=bias_sb[0:1, :],
                        start=False,
                        stop=True,
                    )
                    nc.scalar.activation(
                        out=osb_w[:, j, :],
                        in_=acc[:],
                        func=mybir.ActivationFunctionType.Copy,
                        scale=dinv_sb[:, bb : bb + 1],
                    )
                nc.sync.dma_start(
                    out=out_ext[v * wblk * P : (v + 1) * wblk * P, :].rearrange(
                        "(j p) f -> p j f", p=P
                    ),
                    in_=osb_w[:],
                )

    nc.finalize()
    return nc


def _run(inputs, trace=False, trace_cores=None):
    from concourse.bass_utils import run_bass_kernel_spmd

    meta, in_maps = _pack(**inputs)
    nc = None
    for mb, dk in ((12, 8), (10, 8), (10, 6), (8, 6), (8, 4), (6, 4), (6, 2), (4, 2)):
        try:
            nc = _build_program(meta, mbufs=mb, dskew=dk)
            break
        except ValueError:
            continue
    assert nc is not None
    res = run_bass_kernel_spmd(
        nc,
        in_maps,
        list(range(NCORES)),
        trace=trace,
        trace_cores=trace_cores,
    )
    n, nb, nbp = meta["n"], meta["nb"], meta["nbp"]
    out_new = np.empty((n, P), np.float32)
    for k in range(NCORES):
        out_new[k * nb : (k + 1) * nb] = np.asarray(res.results[k]["out"])[:nb]
    out = out_new[meta["newid"]]
    return out, res


def kernel(x, edge_index, weight, b):
    out, _ = _run(dict(x=x, edge_index=edge_index, weight=weight, b=b))
    return out


if __name__ == "__main__":
    rng = np.random.default_rng(0)
    n, e = 100000, 1600000
    x = rng.standard_normal((n, P), dtype=np.float32)
    ei = rng.integers(0, n, (2, e)).astype(np.int64)
    w = (rng.standard_normal((P, P)) / np.sqrt(P)).astype(np.float32)
    bb = (rng.standard_normal(P) * 0.02).astype(np.float32)
    out = kernel(x, ei, w, bb)
    print("out", out.shape, out.dtype)


# revision 29
# speedup vs baseline: 1.1921x; 1.0045x over previous
"""GCNConv (PyG-faithful, normalize=True, add_self_loops=True) on 8 Trainium2
NeuronCores via Bass/Tile.

Strategy (1D graph/data parallel):
  - Nodes are partitioned across the 8 cores (12500 rows each, padded to
    12544 = 98 blocks of 128).
  - Phase A: each core computes h_k = x_k @ W (fp32 matmuls), scales rows by
    dinv (symmetric GCN normalization, computed host-side from the edge
    index), casts to bf16 and AllGathers the scaled table
    g = dinv[:,None] * (x @ W) into every core's DRAM. The AllGather is
    split in two halves (first/second half of each shard) so phase-B
    gathers on chunks 0-1 can start while the second half is in flight.
  - Phase B: each core owns 1/8 of the destination nodes. Self-loop
    messages are the core's OWN h_shard rows: they are loaded per dst block
    with one affine DMA (no gather). Non-self edges are host-sorted by
    destination block; per 128-edge tile a dma_gather (SWDGE, 4 queues,
    calls split in halves and interleaved across queues to keep all rings
    fed) fetches g[src] rows (bf16). The one-hot selection tiles that map
    each edge to its dst-local row are host-precomputed (fp8) and streamed
    per window as one large-descriptor HWDGE DMA. TensorE segment-sums
    messages into a per-block PSUM accumulator; a final K=1 rank-1 matmul
    (sqrt(deg)[d] (x) bias[f]) folds the output bias into the accumulation
    so the epilogue is a single ScalarE copy scaled by dinv_dst.

  Per-(block, chunk) tile counts are computed from the actual edge data at
  call time (the program is compiled per call), maxed across cores so all 8
  cores run an identical (SPMD) program.
"""

import sys

if "/opt/trn_rl_repo" not in sys.path:
    sys.path.insert(0, "/opt/trn_rl_repo")

import numpy as np

P = 128          # partitions / tile edge count / feature dim
NCORES = 8
WBLK = 2         # blocks per window
CHUNKS = 4       # src chunks for int16 gather indices
GSPLIT = 1       # sub-calls per (window, chunk) gather section
DMA_SCRATCH = 16384  # SWDGE descriptor-ring carveout per partition (default;
                     # 48K/128K were tried and did not move the gather rate)

_PAD_DL = 160.0  # sentinel dst_local for pad edges -> all-zero sel column


def _pack(x, edge_index, weight, b):
    """Host-side preprocessing: sharding, normalization metadata, gather
    index packing, dst-local strips. All numpy, vectorized."""

    bias = b
    x = np.ascontiguousarray(np.asarray(x, dtype=np.float32))
    ei = np.asarray(edge_index)
    weight = np.ascontiguousarray(np.asarray(weight, dtype=np.float32))
    bias = np.asarray(bias, dtype=np.float32).reshape(-1)

    n, nin = x.shape
    nout = weight.shape[1]
    assert nin == P and nout == P, (nin, nout)
    assert n % NCORES == 0, n
    nb = n // NCORES                      # nodes per core (12500)
    blocks = (nb + P - 1) // P            # blocks per core (98)
    nbp = blocks * P                      # padded nodes per core (12544)
    # asymmetric AllGather halves: half A gets ~54% of rows so the early-
    # starting chunk-0/1 gather queues (skewed pipeline, below) carry
    # proportionally more work and all four queues finish together
    ha = 7232                             # shard rows in half A (57.7%)
    hb = nbp - ha                         # shard rows in half B (5760)
    wblk = WBLK if blocks % WBLK == 0 else 1
    nwin = blocks // wblk                 # windows (14)
    crows_a = NCORES * ha // 2            # rows per A chunk (27136)
    crows_b = NCORES * hb // 2            # rows per B chunk (23040)
    assert crows_a < 32768 and crows_b < 32768, (crows_a, crows_b)

    src0 = ei[0].astype(np.int64)
    dst0 = ei[1].astype(np.int64)

    # Degree-balanced relabeling: assign nodes to (core, block, lane) slots
    # snake-wise by in-degree so every block has a near-equal edge count.
    # This shrinks the cross-core max and the ceil-to-128 padding of the
    # per-(block, chunk) gather tiles. Src and dst sides (x, table, output)
    # share the permutation; the host unscrambles the output at the end.
    indeg = np.bincount(dst0, minlength=n)
    by_deg = np.argsort(-indeg, kind="stable")
    nslot = NCORES * blocks               # 784 (core, block) slots
    lastcap = nb - (blocks - 1) * P       # real lanes in the last block (84)
    slot_b = np.arange(nslot) % blocks
    newid = np.empty(n, np.int64)
    ptr = 0
    fwd = np.arange(nslot)
    for lane in range(P):
        active = fwd[(slot_b < blocks - 1) | (lane < lastcap)]
        if lane % 2 == 1:
            active = active[::-1]
        ids = (active // blocks) * nb + (active % blocks) * P + lane
        newid[by_deg[ptr : ptr + active.shape[0]]] = ids
        ptr += active.shape[0]
    assert ptr == n

    src = newid[src0]
    dst = newid[dst0]

    deg = np.bincount(dst, minlength=n).astype(np.float32) + 1.0
    dinv = 1.0 / np.sqrt(deg)
    sdeg = np.sqrt(deg)

    x = x[np.argsort(newid)]              # x row v_new = old row with newid==v_new

    m = src.shape[0]

    core = dst // nb
    dlc = dst - core * nb                 # dst local to core
    blk = dlc >> 7
    dl = (dlc & 127).astype(np.int64)
    # src table row in the split-AllGather layout: table A holds each
    # shard's rows [0, ha), table B rows [ha, nbp)
    kk = src // nb
    r = src % nb
    in_b = r >= ha
    gh = np.where(in_b, kk * hb + (r - ha), kk * ha + r)
    crw = np.where(in_b, crows_b, crows_a)
    chunk = np.where(in_b, 2, 0) + gh // crw
    rel = (gh % crw).astype(np.int16)

    key = (core * blocks + blk) * CHUNKS + chunk
    # sort each section's edges by ascending src row (DRAM locality)
    order = np.lexsort((gh, key))
    karr = core[order]
    relarr = rel[order]
    dlarr = dl[order]
    gkey = key[order]

    counts = np.bincount(key, minlength=NCORES * blocks * CHUNKS).reshape(
        NCORES, blocks, CHUNKS
    )
    t_bc = -(-counts.max(axis=0) // P)    # [blocks, CHUNKS] tiles per slot

    # gather-tile layout: for w in windows: for c in chunks: for b in window
    tile_off = np.zeros((blocks, CHUNKS), np.int64)
    wbase = np.zeros(nwin + 1, np.int64)
    col = 0
    for w in range(nwin):
        wbase[w] = col
        for c in range(CHUNKS):
            for bb in range(w * wblk, (w + 1) * wblk):
                tile_off[bb, c] = col
                col += t_bc[bb, c]
    t_total = int(col)
    wbase[nwin] = col

    # scatter edges into per-core packed arrays
    gs = np.zeros(NCORES * blocks * CHUNKS, np.int64)
    gs[1:] = np.cumsum(counts.ravel())[:-1]
    rank = np.arange(m, dtype=np.int64) - gs[gkey]
    base_flat = (tile_off * P).ravel()    # same for all cores
    dest = base_flat[(gkey % (blocks * CHUNKS))] + rank

    idx_lin = np.zeros((NCORES, t_total * P), np.int16)
    dl_lin = np.full((NCORES, t_total * P), _PAD_DL, np.int16)
    idx_lin[karr, dest] = relarr
    dl_lin[karr, dest] = dlarr.astype(np.int16)

    # wrap-16 + replicate to 128 partitions for dma_gather idx layout
    l16 = t_total * P // 16
    idx_w = idx_lin.reshape(NCORES, l16, 16).transpose(0, 2, 1)  # [8,16,L16]
    idx_pack = np.ascontiguousarray(np.tile(idx_w, (1, NCORES, 1)))  # [8,128,L16]

    # host-precomputed one-hot sel tiles (fp8) for the gather slots, streamed
    # per window over HWDGE (building them on DVE stalls SWDGE descriptor
    # generation: DVE perf-mode ops hold the shared SBUF port pair that
    # GPSIMD needs to write DMA descriptors). sel[e, gt, d] = (dl[gt,e]==d).
    import ml_dtypes

    sel_pack = np.empty((NCORES, P, t_total * P), ml_dtypes.float8_e4m3)
    dgrid = np.arange(P, dtype=np.int16)[None, None, :]
    for k in range(NCORES):
        dlr = dl_lin[k].reshape(t_total, P)          # [gt, e]
        sel_k = dlr.T[:, :, None] == dgrid           # [e, gt, d] bool
        sel_pack[k] = sel_k.reshape(P, t_total * P).astype(ml_dtypes.float8_e4m3)

    # identity tile (fp8) for the self-loop matmuls
    ident_t = np.ascontiguousarray(
        np.eye(P, dtype=np.float32).astype(ml_dtypes.float8_e4m3)
    )

    # per-core xT, dinv, sqrt(deg) row strip (for the K=1 bias matmul)
    import ml_dtypes as _mld
    xt = np.zeros((NCORES, P, nbp), _mld.bfloat16)
    dinv_t = np.zeros((NCORES, P, blocks), np.float32)
    sdeg_t = np.zeros((NCORES, 1, nbp), np.float32)
    for k in range(NCORES):
        xs = x[k * nb : (k + 1) * nb]
        xt[k, :, :nb] = xs.T
        dv = np.zeros(nbp, np.float32)
        dv[:nb] = dinv[k * nb : (k + 1) * nb]
        dinv_t[k] = dv.reshape(blocks, P).T
        sv = np.zeros(nbp, np.float32)
        sv[:nb] = sdeg[k * nb : (k + 1) * nb]
        sdeg_t[k] = sv[None, :]
    bias_rep = np.ascontiguousarray(np.tile(bias[None, :], (P, 1)))

    meta = dict(
        n=n, nb=nb, blocks=blocks, nbp=nbp, nwin=nwin, wblk=wblk, newid=newid,
        ha=ha, hb=hb, crows_a=crows_a, crows_b=crows_b, t_bc=t_bc, tile_off=tile_off,
        wbase=wbase, t_total=t_total, l16=l16,
    )
    in_maps = [
        {
            "xt": xt[k],
            "w_in": weight,
            "bias": bias_rep,
            "dinv": dinv_t[k],
            "sdegT": sdeg_t[k],
            "idxp": idx_pack[k],
            "selp": sel_pack[k],
            "ident": ident_t,
        }
        for k in range(NCORES)
    ]
    return meta, in_maps


def _install_walrus_scratch_flag():
    """Make the walrus backend allocate the same enlarged dynamic-DMA
    scratch carveout that Bacc reserves (the ring size is a compiler flag,
    not a BIR attribute)."""
    from concourse import bass_utils

    if getattr(bass_utils, "_gcn_scratch_patched", None) == DMA_SCRATCH:
        return
    orig = bass_utils.get_walrus_args

    def patched(*args, **kwargs):
        return list(orig(*args, **kwargs)) + [
            f"--dynamic-dma-scratch-size-per-partition={DMA_SCRATCH}"
        ]

    bass_utils.get_walrus_args = patched
    bass_utils._gcn_scratch_patched = DMA_SCRATCH


def _build_program(meta, mbufs=12, dskew=8):
    from concourse import bass, bacc, mybir
    import concourse.tile as tile

    _install_walrus_scratch_flag()

    blocks = meta["blocks"]
    nbp = meta["nbp"]
    ha, hb = meta["ha"], meta["hb"]
    crows = [meta["crows_a"], meta["crows_a"], meta["crows_b"], meta["crows_b"]]
    nwin = meta["nwin"]
    wblk = meta["wblk"]
    t_bc = meta["t_bc"]
    tile_off = meta["tile_off"]
    wbase = meta["wbase"]
    t_total = meta["t_total"]
    l16 = meta["l16"]
    jmax = int((wbase[1:] - wbase[:-1]).max())       # gather tiles per window
    smax = wblk + jmax                               # msg slots per window

    f32 = mybir.dt.float32
    bf16 = mybir.dt.bfloat16
    fp8 = mybir.dt.float8e4

    nc = bacc.Bacc(num_swdge_queues=4, dynamic_dma_scratch_size=DMA_SCRATCH)
    xt_in = nc.declare_dram_parameter("xt", [P, nbp], bf16, isOutput=False)
    w_in = nc.declare_dram_parameter("w_in", [P, P], f32, isOutput=False)
    bias_in = nc.declare_dram_parameter("bias", [P, P], f32, isOutput=False)
    dinv_in = nc.declare_dram_parameter("dinv", [P, blocks], f32, isOutput=False)
    sdegT_in = nc.declare_dram_parameter("sdegT", [1, nbp], f32, isOutput=False)
    idx_in = nc.declare_dram_parameter("idxp", [P, l16], mybir.dt.int16, isOutput=False)
    sel_in = nc.declare_dram_parameter("selp", [P, t_total * P], fp8, isOutput=False)
    ident_in = nc.declare_dram_parameter("ident", [P, P], fp8, isOutput=False)
    out_ext = nc.declare_dram_parameter("out", [nbp, P], f32, isOutput=True)

    h_shard = nc.dram_tensor("h_shard", [nbp, P], bf16)
    g_a = nc.dram_tensor("g_a", [NCORES * ha, P], bf16, addr_space="Shared")
    g_b = nc.dram_tensor("g_b", [NCORES * hb, P], bf16, addr_space="Shared")
    g_half = [g_a, g_a, g_b, g_b]

    with tile.TileContext(nc) as tc:
        with (
            tc.tile_pool(name="const", bufs=1) as cpool,
            tc.tile_pool(name="msgp", bufs=mbufs) as mpool,
            tc.tile_pool(name="selp", bufs=6) as spool,
            tc.tile_pool(name="idxp", bufs=mbufs) as ipool,
            tc.tile_pool(name="outp", bufs=3) as opool,
            tc.tile_pool(name="psB", bufs=6, space="PSUM") as psB,
        ):
            # constants / metadata loads
            w_sb = cpool.tile([P, P], f32, tag="w")
            nc.sync.dma_start(out=w_sb[:], in_=w_in[:])
            w_bf = cpool.tile([P, P], bf16, tag="wbf")
            nc.vector.tensor_scalar(
                out=w_bf[:], in0=w_sb[:], scalar1=1.0, scalar2=None,
                op0=mybir.AluOpType.mult,
            )
            bias_sb = cpool.tile([P, P], f32, tag="bias")
            nc.sync.dma_start(out=bias_sb[:], in_=bias_in[:])
            dinv_sb = cpool.tile([P, blocks], f32, tag="dinv")
            nc.sync.dma_start(out=dinv_sb[:], in_=dinv_in[:])
            ident_sb = cpool.tile([P, P], fp8, tag="ident")
            nc.sync.dma_start(out=ident_sb[:], in_=ident_in[:])

            # ---- phase A: h = x @ W, scale by dinv, cast bf16, allgather
            # (two halves: AG1 covers shard rows [0, half), AG2 the rest)
            with (
                tc.tile_pool(name="workA", bufs=2) as wpool,
                tc.tile_pool(name="psA", bufs=2, space="PSUM") as psA,
            ):
                nchunk = 14
                cw = nbp // nchunk        # nodes per chunk (896)
                tpc = cw // P             # tiles per chunk (7)
                for ch in range(nchunk):
                    xt_t = wpool.tile([P, cw], bf16, tag="xt")
                    nc.sync.dma_start(
                        out=xt_t[:], in_=xt_in[:, ch * cw : (ch + 1) * cw]
                    )
                    hbig = wpool.tile([P, tpc, P], bf16, tag="hbig")
                    for t in range(tpc):
                        ph = psA.tile([P, P], f32, tag="ph")
                        nc.tensor.matmul(
                            out=ph[:],
                            lhsT=xt_t[:, t * P : (t + 1) * P],
                            rhs=w_bf[:],
                            start=True,
                            stop=True,
                        )
                        gb = ch * tpc + t
                        nc.vector.tensor_scalar(
                            out=hbig[:, t, :],
                            in0=ph[:],
                            scalar1=dinv_sb[:, gb : gb + 1],
                            scalar2=None,
                            op0=mybir.AluOpType.mult,
                        )
                    nc.sync.dma_start(
                        out=h_shard[ch * cw : (ch + 1) * cw, :].rearrange(
                            "(t p) f -> p t f", p=P
                        ),
                        in_=hbig[:],
                    )
                    if (ch + 1) * cw >= ha > ch * cw:
                        nc.gpsimd.collective_compute(
                            "AllGather",
                            mybir.AluOpType.bypass,
                            replica_groups=[list(range(NCORES))],
                            ins=[h_shard[0:ha, :]],
                            outs=[g_a[:]],
                        )
                nc.gpsimd.collective_compute(
                    "AllGather",
                    mybir.AluOpType.bypass,
                    replica_groups=[list(range(NCORES))],
                    ins=[h_shard[ha:nbp, :]],
                    outs=[g_b[:]],
                )

            # ---- phase B: skewed pipeline. Chunk-0/1 gathers (table half A,
            # ready after AG1) issue for window w while chunk-2/3 gathers
            # (need AG2) issue for window w-dskew: AG2-blocked calls then
            # never clog the Pool engine's 4-deep wait queue while ready
            # chunk-0/1 work exists. Matmuls/epilogue run at w-dskew.
            state = {}

            def sections_of(w):
                secs = []
                for c in range(CHUNKS):
                    sec0 = None
                    seclen = 0
                    for bb in range(w * wblk, (w + 1) * wblk):
                        if t_bc[bb, c] > 0:
                            if sec0 is None:
                                sec0 = int(tile_off[bb, c])
                            seclen += int(t_bc[bb, c])
                    secs.append((sec0, seclen))
                return secs

            def emit_gather(w, c, idx_t, msg):
                sec0, seclen = state[w]["secs"][c]
                if seclen == 0:
                    return
                lo = sec0 - int(wbase[w])
                nc.gpsimd.dma_gather(
                    out_ap=msg[:, wblk + lo : wblk + lo + seclen, :],
                    in_ap=g_half[c][
                        (c % 2) * crows[c] : (c % 2 + 1) * crows[c], :
                    ],
                    idxs_ap=idx_t[:, lo * 8 : (lo + seclen) * 8],
                    num_idxs=seclen * P,
                    num_idxs_reg=seclen * P,
                    elem_size=P,
                    single_packet=False,
                    queue_num=c,
                )

            for step in range(nwin + dskew):
                if step < nwin:
                    w = step
                    jsize = int(wbase[w + 1] - wbase[w])
                    idx_t = ipool.tile([P, jmax * 8], mybir.dt.int16, tag="idxw")
                    if jsize > 0:
                        nc.scalar.dma_start(
                            out=idx_t[:, : jsize * 8],
                            in_=idx_in[:, int(wbase[w]) * 8 : int(wbase[w + 1]) * 8],
                        )
                    msg = mpool.tile([P, smax, P], bf16, tag="msg")
                    nc.sync.dma_start(
                        out=msg[:, 0:wblk, :],
                        in_=h_shard[w * wblk * P : (w + 1) * wblk * P, :].rearrange(
                            "(j p) f -> p j f", p=P
                        ),
                    )
                    state[w] = dict(secs=sections_of(w), idx=idx_t, msg=msg, jsize=jsize)
                    emit_gather(w, 0, idx_t, msg)
                    emit_gather(w, 1, idx_t, msg)
                v = step - dskew
                if not (0 <= v < nwin):
                    continue
                st = state[v]
                idx_t, msg, jsize = st["idx"], st["msg"], st["jsize"]
                emit_gather(v, 2, idx_t, msg)
                emit_gather(v, 3, idx_t, msg)
                del state[v]
                selw = spool.tile([P, jmax, P], fp8, tag="selw")
                if jsize > 0:
                    nc.scalar.dma_start(
                        out=selw[:, :jsize, :],
                        in_=sel_in[:, int(wbase[v]) * P : int(wbase[v + 1]) * P],
                    )
                sdw = opool.tile([1, wblk * P], f32, tag="sdw")
                nc.sync.dma_start(
                    out=sdw[:],
                    in_=sdegT_in[:, v * wblk * P : (v + 1) * wblk * P],
                )
                osb_w = opool.tile([P, wblk, P], f32, tag="osbw")
                for j, bb in enumerate(range(v * wblk, (v + 1) * wblk)):
                    acc = psB.tile([P, P], f32, tag="acc")
                    nc.tensor.matmul(
                        out=acc[:],
                        lhsT=ident_sb[:],
                        rhs=msg[:, j, :],
                        start=True,
                        stop=False,
                    )
                    for c in range(CHUNKS):
                        tb = int(t_bc[bb, c])
                        for t in range(tb):
                            gt = int(tile_off[bb, c]) + t
                            scol = gt - int(wbase[v])
                            nc.tensor.matmul(
                                out=acc[:],
                                lhsT=selw[:, scol, :],
                                rhs=msg[:, wblk + scol, :],
                                start=False,
                                stop=False,
                            )
                    nc.tensor.matmul(
                        out=acc[:],
                        lhsT=sdw[:, j * P : (j + 1) * P],
                        rhs=bias_sb[0:1, :],
                        start=False,
                        stop=True,
                    )
                    nc.scalar.activation(
                        out=osb_w[:, j, :],
                        in_=acc[:],
                        func=mybir.ActivationFunctionType.Copy,
                        scale=dinv_sb[:, bb : bb + 1],
                    )
                nc.sync.dma_start(
                    out=out_ext[v * wblk * P : (v + 1) * wblk * P, :].rearrange(
                        "(j p) f -> p j f", p=P
                    ),
                    in_=osb_w[:],
                )

    nc.finalize()
    return nc


def _run(inputs, trace=False, trace_cores=None):
    from concourse.bass_utils import run_bass_kernel_spmd

    meta, in_maps = _pack(**inputs)
    nc = None
    for mb, dk in ((12, 8), (10, 8), (10, 6), (8, 6), (8, 4), (6, 4), (6, 2), (4, 2)):
        try:
            nc = _build_program(meta, mbufs=mb, dskew=dk)
            break
        except ValueError:
            continue
    assert nc is not None
    res = run_bass_kernel_spmd(
        nc,
        in_maps,
        list(range(NCORES)),
        trace=trace,
        trace_cores=trace_cores,
    )
    n, nb, nbp = meta["n"], meta["nb"], meta["nbp"]
    out_new = np.empty((n, P), np.float32)
    for k in range(NCORES):
        out_new[k * nb : (k + 1) * nb] = np.asarray(res.results[k]["out"])[:nb]
    out = out_new[meta["newid"]]
    return out, res


def kernel(x, edge_index, weight, b):
    out, _ = _run(dict(x=x, edge_index=edge_index, weight=weight, b=b))
    return out


if __name__ == "__main__":
    rng = np.random.default_rng(0)
    n, e = 100000, 1600000
    x = rng.standard_normal((n, P), dtype=np.float32)
    ei = rng.integers(0, n, (2, e)).astype(np.int64)
    w = (rng.standard_normal((P, P)) / np.sqrt(P)).astype(np.float32)
    bb = (rng.standard_normal(P) * 0.02).astype(np.float32)
    out = kernel(x, ei, w, bb)
    print("out", out.shape, out.dtype)


# revision 31
# speedup vs baseline: 1.2136x; 1.0181x over previous
"""GCNConv (PyG-faithful, normalize=True, add_self_loops=True) on 8 Trainium2
NeuronCores via Bass/Tile.

Strategy (1D graph/data parallel):
  - Nodes are partitioned across the 8 cores (12500 rows each, padded to
    12544 = 98 blocks of 128).
  - Phase A: each core computes h_k = x_k @ W (fp32 matmuls), scales rows by
    dinv (symmetric GCN normalization, computed host-side from the edge
    index), casts to bf16 and AllGathers the scaled table
    g = dinv[:,None] * (x @ W) into every core's DRAM. The AllGather is
    split in two halves (first/second half of each shard) so phase-B
    gathers on chunks 0-1 can start while the second half is in flight.
  - Phase B: each core owns 1/8 of the destination nodes. Self-loop
    messages are the core's OWN h_shard rows: they are loaded per dst block
    with one affine DMA (no gather). Non-self edges are host-sorted by
    destination block; per 128-edge tile a dma_gather (SWDGE, 4 queues,
    calls split in halves and interleaved across queues to keep all rings
    fed) fetches g[src] rows (bf16). The one-hot selection tiles that map
    each edge to its dst-local row are host-precomputed (fp8) and streamed
    per window as one large-descriptor HWDGE DMA. TensorE segment-sums
    messages into a per-block PSUM accumulator; a final K=1 rank-1 matmul
    (sqrt(deg)[d] (x) bias[f]) folds the output bias into the accumulation
    so the epilogue is a single ScalarE copy scaled by dinv_dst.

  Per-(block, chunk) tile counts are computed from the actual edge data at
  call time (the program is compiled per call), maxed across cores so all 8
  cores run an identical (SPMD) program.
"""

import sys

if "/opt/trn_rl_repo" not in sys.path:
    sys.path.insert(0, "/opt/trn_rl_repo")

import numpy as np

P = 128          # partitions / tile edge count / feature dim
NCORES = 8
WBLK = 2         # blocks per window
CHUNKS = 4       # src chunks for int16 gather indices
GSPLIT = 1       # sub-calls per (window, chunk) gather section
DMA_SCRATCH = 16384  # SWDGE descriptor-ring carveout per partition (default;
                     # 48K/128K were tried and did not move the gather rate)

_PAD_DL = 160.0  # sentinel dst_local for pad edges -> all-zero sel column


def _pack(x, edge_index, weight, b):
    """Host-side preprocessing: sharding, normalization metadata, gather
    index packing, dst-local strips. All numpy, vectorized."""

    bias = b
    x = np.ascontiguousarray(np.asarray(x, dtype=np.float32))
    ei = np.asarray(edge_index)
    weight = np.ascontiguousarray(np.asarray(weight, dtype=np.float32))
    bias = np.asarray(bias, dtype=np.float32).reshape(-1)

    n, nin = x.shape
    nout = weight.shape[1]
    assert nin == P and nout == P, (nin, nout)
    assert n % NCORES == 0, n
    nb = n // NCORES                      # nodes per core (12500)
    blocks = (nb + P - 1) // P            # blocks per core (98)
    nbp = blocks * P                      # padded nodes per core (12544)
    # asymmetric AllGather halves: half A gets ~54% of rows so the early-
    # starting chunk-0/1 gather queues (skewed pipeline, below) carry
    # proportionally more work and all four queues finish together
    ha = 7232                             # shard rows in half A (57.7%)
    hb = nbp - ha                         # shard rows in half B (5760)
    wblk = WBLK if blocks % WBLK == 0 else 1
    nwin = blocks // wblk                 # windows (14)
    crows_a = NCORES * ha // 2            # rows per A chunk (27136)
    crows_b = NCORES * hb // 2            # rows per B chunk (23040)
    assert crows_a < 32768 and crows_b < 32768, (crows_a, crows_b)

    src0 = ei[0].astype(np.int64)
    dst0 = ei[1].astype(np.int64)

    # Degree-balanced relabeling: assign nodes to (core, block, lane) slots
    # snake-wise by in-degree so every block has a near-equal edge count.
    # This shrinks the cross-core max and the ceil-to-128 padding of the
    # per-(block, chunk) gather tiles. Src and dst sides (x, table, output)
    # share the permutation; the host unscrambles the output at the end.
    indeg = np.bincount(dst0, minlength=n)
    by_deg = np.argsort(-indeg, kind="stable")
    nslot = NCORES * blocks               # 784 (core, block) slots
    lastcap = nb - (blocks - 1) * P       # real lanes in the last block (84)
    slot_b = np.arange(nslot) % blocks
    newid = np.empty(n, np.int64)
    ptr = 0
    fwd = np.arange(nslot)
    for lane in range(P):
        active = fwd[(slot_b < blocks - 1) | (lane < lastcap)]
        if lane % 2 == 1:
            active = active[::-1]
        ids = (active // blocks) * nb + (active % blocks) * P + lane
        newid[by_deg[ptr : ptr + active.shape[0]]] = ids
        ptr += active.shape[0]
    assert ptr == n

    src = newid[src0]
    dst = newid[dst0]

    deg = np.bincount(dst, minlength=n).astype(np.float32) + 1.0
    dinv = 1.0 / np.sqrt(deg)
    sdeg = np.sqrt(deg)

    x = x[np.argsort(newid)]              # x row v_new = old row with newid==v_new

    m = src.shape[0]

    core = dst // nb
    dlc = dst - core * nb                 # dst local to core
    blk = dlc >> 7
    dl = (dlc & 127).astype(np.int64)
    # src table row in the split-AllGather layout: table A holds each
    # shard's rows [0, ha), table B rows [ha, nbp)
    kk = src // nb
    r = src % nb
    in_b = r >= ha
    gh = np.where(in_b, kk * hb + (r - ha), kk * ha + r)
    crw = np.where(in_b, crows_b, crows_a)
    chunk = np.where(in_b, 2, 0) + gh // crw
    rel = (gh % crw).astype(np.int16)

    key = (core * blocks + blk) * CHUNKS + chunk
    # sort each section's edges by ascending src row (DRAM locality)
    order = np.lexsort((gh, key))
    karr = core[order]
    relarr = rel[order]
    dlarr = dl[order]
    gkey = key[order]

    counts = np.bincount(key, minlength=NCORES * blocks * CHUNKS).reshape(
        NCORES, blocks, CHUNKS
    )
    t_bc = -(-counts.max(axis=0) // P)    # [blocks, CHUNKS] tiles per slot

    # gather-tile layout: for w in windows: for c in chunks: for b in window
    tile_off = np.zeros((blocks, CHUNKS), np.int64)
    wbase = np.zeros(nwin + 1, np.int64)
    col = 0
    for w in range(nwin):
        wbase[w] = col
        for c in range(CHUNKS):
            for bb in range(w * wblk, (w + 1) * wblk):
                tile_off[bb, c] = col
                col += t_bc[bb, c]
    t_total = int(col)
    wbase[nwin] = col

    # scatter edges into per-core packed arrays
    gs = np.zeros(NCORES * blocks * CHUNKS, np.int64)
    gs[1:] = np.cumsum(counts.ravel())[:-1]
    rank = np.arange(m, dtype=np.int64) - gs[gkey]
    base_flat = (tile_off * P).ravel()    # same for all cores
    dest = base_flat[(gkey % (blocks * CHUNKS))] + rank

    idx_lin = np.zeros((NCORES, t_total * P), np.int16)
    dl_lin = np.full((NCORES, t_total * P), _PAD_DL, np.int16)
    idx_lin[karr, dest] = relarr
    dl_lin[karr, dest] = dlarr.astype(np.int16)

    # wrap-16 + replicate to 128 partitions for dma_gather idx layout
    l16 = t_total * P // 16
    idx_w = idx_lin.reshape(NCORES, l16, 16).transpose(0, 2, 1)  # [8,16,L16]
    idx_pack = np.ascontiguousarray(np.tile(idx_w, (1, NCORES, 1)))  # [8,128,L16]

    # host-precomputed one-hot sel tiles (fp8) for the gather slots, streamed
    # per window over HWDGE (building them on DVE stalls SWDGE descriptor
    # generation: DVE perf-mode ops hold the shared SBUF port pair that
    # GPSIMD needs to write DMA descriptors). sel[e, gt, d] = (dl[gt,e]==d).
    import ml_dtypes

    sel_pack = np.empty((NCORES, P, t_total * P), ml_dtypes.float8_e4m3)
    dgrid = np.arange(P, dtype=np.int16)[None, None, :]
    for k in range(NCORES):
        dlr = dl_lin[k].reshape(t_total, P)          # [gt, e]
        sel_k = dlr.T[:, :, None] == dgrid           # [e, gt, d] bool
        sel_pack[k] = sel_k.reshape(P, t_total * P).astype(ml_dtypes.float8_e4m3)

    # identity tile (fp8) for the self-loop matmuls
    ident_t = np.ascontiguousarray(
        np.eye(P, dtype=np.float32).astype(ml_dtypes.float8_e4m3)
    )

    # per-core xT, dinv, sqrt(deg) row strip (for the K=1 bias matmul)
    import ml_dtypes as _mld
    xt = np.zeros((NCORES, P, nbp), _mld.bfloat16)
    dinv_t = np.zeros((NCORES, P, blocks), np.float32)
    sdeg_t = np.zeros((NCORES, 1, nbp), np.float32)
    for k in range(NCORES):
        xs = x[k * nb : (k + 1) * nb]
        xt[k, :, :nb] = xs.T
        dv = np.zeros(nbp, np.float32)
        dv[:nb] = dinv[k * nb : (k + 1) * nb]
        dinv_t[k] = dv.reshape(blocks, P).T
        sv = np.zeros(nbp, np.float32)
        sv[:nb] = sdeg[k * nb : (k + 1) * nb]
        sdeg_t[k] = sv[None, :]
    bias_rep = np.ascontiguousarray(np.tile(bias[None, :], (P, 1)))

    meta = dict(
        n=n, nb=nb, blocks=blocks, nbp=nbp, nwin=nwin, wblk=wblk, newid=newid,
        ha=ha, hb=hb, crows_a=crows_a, crows_b=crows_b, t_bc=t_bc, tile_off=tile_off,
        wbase=wbase, t_total=t_total, l16=l16,
    )
    in_maps = [
        {
            "xt": xt[k],
            "w_in": weight,
            "bias": bias_rep,
            "dinv": dinv_t[k],
            "sdegT": sdeg_t[k],
            "idxp": idx_pack[k],
            "selp": sel_pack[k],
            "ident": ident_t,
        }
        for k in range(NCORES)
    ]
    return meta, in_maps


def _install_walrus_scratch_flag():
    """Make the walrus backend allocate the same enlarged dynamic-DMA
    scratch carveout that Bacc reserves (the ring size is a compiler flag,
    not a BIR attribute)."""
    from concourse import bass_utils

    if getattr(bass_utils, "_gcn_scratch_patched", None) == DMA_SCRATCH:
        return
    orig = bass_utils.get_walrus_args

    def patched(*args, **kwargs):
        return list(orig(*args, **kwargs)) + [
            f"--dynamic-dma-scratch-size-per-partition={DMA_SCRATCH}"
        ]

    bass_utils.get_walrus_args = patched
    bass_utils._gcn_scratch_patched = DMA_SCRATCH


def _build_program(meta, mbufs=12, dskew=8):
    from concourse import bass, bacc, mybir
    import concourse.tile as tile

    _install_walrus_scratch_flag()

    blocks = meta["blocks"]
    nbp = meta["nbp"]
    ha, hb = meta["ha"], meta["hb"]
    crows = [meta["crows_a"], meta["crows_a"], meta["crows_b"], meta["crows_b"]]
    nwin = meta["nwin"]
    wblk = meta["wblk"]
    t_bc = meta["t_bc"]
    tile_off = meta["tile_off"]
    wbase = meta["wbase"]
    t_total = meta["t_total"]
    l16 = meta["l16"]
    jmax = int((wbase[1:] - wbase[:-1]).max())       # gather tiles per window
    smax = wblk + jmax                               # msg slots per window

    f32 = mybir.dt.float32
    bf16 = mybir.dt.bfloat16
    fp8 = mybir.dt.float8e4

    nc = bacc.Bacc(num_swdge_queues=4, dynamic_dma_scratch_size=DMA_SCRATCH)
    xt_in = nc.declare_dram_parameter("xt", [P, nbp], bf16, isOutput=False)
    w_in = nc.declare_dram_parameter("w_in", [P, P], f32, isOutput=False)
    bias_in = nc.declare_dram_parameter("bias", [P, P], f32, isOutput=False)
    dinv_in = nc.declare_dram_parameter("dinv", [P, blocks], f32, isOutput=False)
    sdegT_in = nc.declare_dram_parameter("sdegT", [1, nbp], f32, isOutput=False)
    idx_in = nc.declare_dram_parameter("idxp", [P, l16], mybir.dt.int16, isOutput=False)
    sel_in = nc.declare_dram_parameter("selp", [P, t_total * P], fp8, isOutput=False)
    ident_in = nc.declare_dram_parameter("ident", [P, P], fp8, isOutput=False)
    out_ext = nc.declare_dram_parameter("out", [nbp, P], f32, isOutput=True)

    h_shard = nc.dram_tensor("h_shard", [nbp, P], bf16)
    g_a = nc.dram_tensor("g_a", [NCORES * ha, P], bf16, addr_space="Shared")
    g_b = nc.dram_tensor("g_b", [NCORES * hb, P], bf16, addr_space="Shared")
    g_half = [g_a, g_a, g_b, g_b]

    with tile.TileContext(nc) as tc:
        with (
            tc.tile_pool(name="const", bufs=1) as cpool,
            tc.tile_pool(name="msgp", bufs=mbufs) as mpool,
            tc.tile_pool(name="selp", bufs=6) as spool,
            tc.tile_pool(name="idxp", bufs=mbufs) as ipool,
            tc.tile_pool(name="outp", bufs=3) as opool,
            tc.tile_pool(name="psB", bufs=6, space="PSUM") as psB,
        ):
            # constants / metadata loads
            w_sb = cpool.tile([P, P], f32, tag="w")
            nc.sync.dma_start(out=w_sb[:], in_=w_in[:])
            w_bf = cpool.tile([P, P], bf16, tag="wbf")
            nc.vector.tensor_scalar(
                out=w_bf[:], in0=w_sb[:], scalar1=1.0, scalar2=None,
                op0=mybir.AluOpType.mult,
            )
            bias_sb = cpool.tile([P, P], f32, tag="bias")
            nc.sync.dma_start(out=bias_sb[:], in_=bias_in[:])
            dinv_sb = cpool.tile([P, blocks], f32, tag="dinv")
            nc.sync.dma_start(out=dinv_sb[:], in_=dinv_in[:])
            ident_sb = cpool.tile([P, P], fp8, tag="ident")
            nc.sync.dma_start(out=ident_sb[:], in_=ident_in[:])

            # ---- phase A: h = x @ W, scale by dinv, cast bf16, allgather
            # (two halves: AG1 covers shard rows [0, half), AG2 the rest)
            with (
                tc.tile_pool(name="workA", bufs=2) as wpool,
                tc.tile_pool(name="psA", bufs=2, space="PSUM") as psA,
            ):
                nchunk = 14
                cw = nbp // nchunk        # nodes per chunk (896)
                tpc = cw // P             # tiles per chunk (7)
                for ch in range(nchunk):
                    xt_t = wpool.tile([P, cw], bf16, tag="xt")
                    nc.sync.dma_start(
                        out=xt_t[:], in_=xt_in[:, ch * cw : (ch + 1) * cw]
                    )
                    hbig = wpool.tile([P, tpc, P], bf16, tag="hbig")
                    for t in range(tpc):
                        ph = psA.tile([P, P], f32, tag="ph")
                        nc.tensor.matmul(
                            out=ph[:],
                            lhsT=xt_t[:, t * P : (t + 1) * P],
                            rhs=w_bf[:],
                            start=True,
                            stop=True,
                        )
                        gb = ch * tpc + t
                        nc.vector.tensor_scalar(
                            out=hbig[:, t, :],
                            in0=ph[:],
                            scalar1=dinv_sb[:, gb : gb + 1],
                            scalar2=None,
                            op0=mybir.AluOpType.mult,
                        )
                    nc.sync.dma_start(
                        out=h_shard[ch * cw : (ch + 1) * cw, :].rearrange(
                            "(t p) f -> p t f", p=P
                        ),
                        in_=hbig[:],
                    )
                    if (ch + 1) * cw >= ha > ch * cw:
                        nc.gpsimd.collective_compute(
                            "AllGather",
                            mybir.AluOpType.bypass,
                            replica_groups=[list(range(NCORES))],
                            ins=[h_shard[0:ha, :]],
                            outs=[g_a[:]],
                        )
                nc.gpsimd.collective_compute(
                    "AllGather",
                    mybir.AluOpType.bypass,
                    replica_groups=[list(range(NCORES))],
                    ins=[h_shard[ha:nbp, :]],
                    outs=[g_b[:]],
                )

            # ---- phase B: skewed pipeline. Chunk-0/1 gathers (table half A,
            # ready after AG1) issue for window w while chunk-2/3 gathers
            # (need AG2) issue for window w-dskew: AG2-blocked calls then
            # never clog the Pool engine's 4-deep wait queue while ready
            # chunk-0/1 work exists. Matmuls/epilogue run at w-dskew.
            state = {}

            def sections_of(w):
                secs = []
                for c in range(CHUNKS):
                    sec0 = None
                    seclen = 0
                    for bb in range(w * wblk, (w + 1) * wblk):
                        if t_bc[bb, c] > 0:
                            if sec0 is None:
                                sec0 = int(tile_off[bb, c])
                            seclen += int(t_bc[bb, c])
                    secs.append((sec0, seclen))
                return secs

            def emit_gather(w, c, idx_t, msg):
                sec0, seclen = state[w]["secs"][c]
                if seclen == 0:
                    return
                lo = sec0 - int(wbase[w])
                nc.gpsimd.dma_gather(
                    out_ap=msg[:, wblk + lo : wblk + lo + seclen, :],
                    in_ap=g_half[c][
                        (c % 2) * crows[c] : (c % 2 + 1) * crows[c], :
                    ],
                    idxs_ap=idx_t[:, lo * 8 : (lo + seclen) * 8],
                    num_idxs=seclen * P,
                    num_idxs_reg=seclen * P,
                    elem_size=P,
                    single_packet=False,
                    queue_num=c,
                )

            for step in range(nwin + dskew):
                if step < nwin:
                    w = step
                    jsize = int(wbase[w + 1] - wbase[w])
                    idx_t = ipool.tile([P, jmax * 8], mybir.dt.int16, tag="idxw")
                    if jsize > 0:
                        nc.scalar.dma_start(
                            out=idx_t[:, : jsize * 8],
                            in_=idx_in[:, int(wbase[w]) * 8 : int(wbase[w + 1]) * 8],
                        )
                    msg = mpool.tile([P, smax, P], bf16, tag="msg")
                    nc.sync.dma_start(
                        out=msg[:, 0:wblk, :],
                        in_=h_shard[w * wblk * P : (w + 1) * wblk * P, :].rearrange(
                            "(j p) f -> p j f", p=P
                        ),
                    )
                    state[w] = dict(secs=sections_of(w), idx=idx_t, msg=msg, jsize=jsize)
                    emit_gather(w, 0, idx_t, msg)
                    emit_gather(w, 1, idx_t, msg)
                v = step - dskew
                if not (0 <= v < nwin):
                    continue
                st = state[v]
                idx_t, msg, jsize = st["idx"], st["msg"], st["jsize"]
                emit_gather(v, 2, idx_t, msg)
                emit_gather(v, 3, idx_t, msg)
                del state[v]
                selw = spool.tile([P, jmax, P], fp8, tag="selw")
                if jsize > 0:
                    nc.scalar.dma_start(
                        out=selw[:, :jsize, :],
                        in_=sel_in[:, int(wbase[v]) * P : int(wbase[v + 1]) * P],
                    )
                sdw = opool.tile([1, wblk * P], f32, tag="sdw")
                nc.sync.dma_start(
                    out=sdw[:],
                    in_=sdegT_in[:, v * wblk * P : (v + 1) * wblk * P],
                )
                osb_w = opool.tile([P, wblk, P], f32, tag="osbw")
                for j, bb in enumerate(range(v * wblk, (v + 1) * wblk)):
                    acc = psB.tile([P, P], f32, tag="acc")
                    nc.tensor.matmul(
                        out=acc[:],
                        lhsT=ident_sb[:],
                        rhs=msg[:, j, :],
                        start=True,
                        stop=False,
                    )
                    for c in range(CHUNKS):
                        tb = int(t_bc[bb, c])
                        for t in range(tb):
                            gt = int(tile_off[bb, c]) + t
                            scol = gt - int(wbase[v])
                            nc.tensor.matmul(
                                out=acc[:],
                                lhsT=selw[:, scol, :],
                                rhs=msg[:, wblk + scol, :],
                                start=False,
                                stop=False,
                            )
                    nc.tensor.matmul(
                        out=acc[:],
                        lhsT=sdw[:, j * P : (j + 1) * P],
                        rhs=bias_sb[0:1, :],
                        start=False,
                        stop=True,
                    )
                    nc.scalar.activation(
                        out=osb_w[:, j, :],
                        in_=acc[:],
                        func=mybir.ActivationFunctionType.Copy,
                        scale=dinv_sb[:, bb : bb + 1],
                    )
                nc.sync.dma_start(
                    out=out_ext[v * wblk * P : (v + 1) * wblk * P, :].rearrange(
                        "(j p) f -> p j f", p=P
                    ),
                    in_=osb_w[:],
                )

    nc.finalize()
    return nc


def _run(inputs, trace=False, trace_cores=None):
    from concourse.bass_utils import run_bass_kernel_spmd

    meta, in_maps = _pack(**inputs)
    nc = None
    for mb, dk in ((12, 8), (10, 8), (10, 6), (8, 6), (8, 4), (6, 4), (6, 2), (4, 2)):
        try:
            nc = _build_program(meta, mbufs=mb, dskew=dk)
            break
        except ValueError:
            continue
    assert nc is not None
    res = run_bass_kernel_spmd(
        nc,
        in_maps,
        list(range(NCORES)),
        trace=trace,
        trace_cores=trace_cores,
    )
    n, nb, nbp = meta["n"], meta["nb"], meta["nbp"]
    out_new = np.empty((n, P), np.float32)
    for k in range(NCORES):
        out_new[k * nb : (k + 1) * nb] = np.asarray(res.results[k]["out"])[:nb]
    out = out_new[meta["newid"]]
    return out, res


def kernel(x, edge_index, weight, b):
    out, _ = _run(dict(x=x, edge_index=edge_index, weight=weight, b=b))
    return out


if __name__ == "__main__":
    rng = np.random.default_rng(0)
    n, e = 100000, 1600000
    x = rng.standard_normal((n, P), dtype=np.float32)
    ei = rng.integers(0, n, (2, e)).astype(np.int64)
    w = (rng.standard_normal((P, P)) / np.sqrt(P)).astype(np.float32)
    bb = (rng.standard_normal(P) * 0.02).astype(np.float32)
    out = kernel(x, ei, w, bb)
    print("out", out.shape, out.dtype)
